# revision 5
# baseline (speedup 1.0000x reference)
"""Trainium2 Bass kernel for SAGAN-style 2D self-attention (nn_Attention2d).

Reference computation (per batch element b):
    q  = query_input[b].reshape(Cq, N)          # N = H*W = 4096, Cq = 256
    kv = key_value_input[b].reshape(C, N)       # C = 256
    fT = Wf @ q + bf        # [32, N]   (f transposed)
    g  = Wg @ kv + bg       # [32, N]
    h  = (Wh @ kv + bh).T   # [N, C]
    beta = softmax(fT.T @ g, axis=-1)           # [N, N]
    o  = beta @ h                               # [N, C]
    out[b] = gamma * o.T + kv                   # [C, N] -> [C, H, W]

Sharding: data-parallel over batch, one batch element per NeuronCore (B=8,
8 cores, no collectives).

Per-core algorithm (all layouts transposed so softmax reductions ride the
matmul path; no on-chip transposes needed):
  - projections: fT [32,N], g [32,N] (token-major), h [N,C] (token, channel)
  - loop over i-tiles (512 query tokens):
      for each j-tile (128 key tokens):
        ST  = g_j^T @ fT_i          # [128 j, 512 i] logits, PSUM
        E   = exp(ST)               # ACT, bf16 -> SBUF (no max subtraction:
                                    #  logits are bounded ~ +-13 for this op)
        o0 += h_j[:, 0:128]^T @ E   # accumulate over j in PSUM  [128 c, 512 i]
        o1 += h_j[:,128:256]^T @ E
        d  += ones^T @ E            # softmax denominator row [1, 512 i]
      s  = gamma / d                # [1, 512]
      bs = ones_col^T @ s           # PE broadcast across partitions
      out_c = o_c * bs + (kv + gamma*bh)   # DVE, then DMA out
"""

import os
import numpy as np

P = 128          # partitions
N = 4096         # tokens (H*W)
CA = 32          # attention channels
C = 256          # kv channels
IT = 512         # i-tile (query tokens per tile)
NI = N // IT     # 8
NJ = N // P      # 32
NCORES = 8

_cache = {}


def _split_multi_waits(nc, keep=1):
    """This walrus build encodes at most one sem wait per instruction
    (setupSyncWait: 'Too many sync wait commands').  Tile's sem assignment
    can attach several.  Move excess waits onto single-wait NoOps emitted
    just before the instruction on the same engine (engines execute their
    stream in order, so the waits still gate the instruction)."""
    import concourse.mybir as mybir
    import bass_rust

    for fn in nc.m.functions:
        for blk in fn.blocks:
            out = []
            for inst in blk.instructions:
                si = inst.sync_info
                if si is not None and len(si.on_wait) > keep:
                    waits = list(si.on_wait)
                    for k, w in enumerate(waits[:-keep]):
                        nop = mybir.InstNoOp(
                            name=f"{inst.name}_prewait{k}", ins=[], outs=[]
                        )
                        nop.engine = inst.engine
                        nop.sync_info = bass_rust.SyncInfo(on_wait=[w], on_update=[])
                        out.append(nop)
                    inst.sync_info = bass_rust.SyncInfo(
                        on_wait=waits[-keep:], on_update=list(si.on_update)
                    )
                out.append(inst)
            blk.instructions = out


def _build():
    import concourse.bass as bass
    import concourse.mybir as mybir
    from concourse.tile import TileContext
    from concourse.bass import ts

    f32 = mybir.dt.float32
    bf16 = mybir.dt.bfloat16
    AF = mybir.ActivationFunctionType

    nc = bass.Bass()
    q_d = nc.dram_tensor("q", [2, P, N], f32, kind="ExternalInput")
    kv_d = nc.dram_tensor("kv", [2, P, N], f32, kind="ExternalInput")
    wft_d = nc.dram_tensor("wft", [2, P, CA], f32, kind="ExternalInput")
    wgt_d = nc.dram_tensor("wgt", [2, P, CA], f32, kind="ExternalInput")
    wht_d = nc.dram_tensor("wht", [2, P, C], f32, kind="ExternalInput")
    bf_d = nc.dram_tensor("bfv", [CA, 1], f32, kind="ExternalInput")
    bg_d = nc.dram_tensor("bgv", [CA, 1], f32, kind="ExternalInput")
    bh_d = nc.dram_tensor("bhv", [P, 2], f32, kind="ExternalInput")
    gm_d = nc.dram_tensor("gam", [1, 1], f32, kind="ExternalInput")
    out_d = nc.dram_tensor("out", [2, P, N], f32, kind="ExternalOutput")

    with TileContext(nc) as tc:
        with (
            tc.tile_pool(name="const", bufs=1) as const,
            tc.tile_pool(name="big", bufs=1) as big,
        ):
            # ---- constants / small params ----
            wft = const.tile([P, 2, CA], f32)
            wgt = const.tile([P, 2, CA], f32)
            wht = const.tile([P, 2, C], f32)
            for k in range(2):
                nc.sync.dma_start(out=wft[:, k, :], in_=wft_d[k, :, :])
                nc.sync.dma_start(out=wgt[:, k, :], in_=wgt_d[k, :, :])
                nc.sync.dma_start(out=wht[:, k, :], in_=wht_d[k, :, :])
            bfs = const.tile([CA, 1], f32)
            bgs = const.tile([CA, 1], f32)
            nc.sync.dma_start(out=bfs, in_=bf_d[:, :])
            nc.sync.dma_start(out=bgs, in_=bg_d[:, :])
            bhs = const.tile([P, 2], f32)
            nc.sync.dma_start(out=bhs, in_=bh_d[:, :])
            gam = const.tile([P, 1], f32)
            nc.sync.dma_start(out=gam, in_=gm_d[:, :].to_broadcast([P, 1]))
            ones_col = const.tile([P, 1], bf16)
            nc.vector.memset(ones_col, 1.0)
            ones_row = const.tile([1, P], f32)
            nc.vector.memset(ones_row, 1.0)

            # gbh = gamma * bh  (per-partition, [128, 2])
            gbh = const.tile([P, 2], f32)
            nc.vector.tensor_scalar_mul(gbh, bhs, gam)

            # ---- big SBUF residents ----
            kv_sb = big.tile([P, 2, N], f32)
            for k in range(2):
                nc.sync.dma_start(out=kv_sb[:, k, :], in_=kv_d[k, :, :])
            kvgb = big.tile([P, 2, N], f32)  # kv + gamma*bh
            fT = big.tile([CA, N], bf16)
            g = big.tile([CA, N], bf16)
            h = big.tile([P, NJ, C], bf16)  # [token-in-jtile, jtile, channel]

            # kvgb = kv + gamma*bh (per-partition bias add on ACT)
            for k in range(2):
                nc.scalar.activation(
                    out=kvgb[:, k, :],
                    in_=kv_sb[:, k, :],
                    func=AF.Identity,
                    bias=gbh[:, k : k + 1],
                )

            # ---- projections ----
            with (
                tc.tile_pool(name="qpool", bufs=1) as qpool,
                tc.tile_pool(name="projps", bufs=2, space="PSUM") as pj,
            ):
                q_sb = qpool.tile([P, 2, N], f32)
                for k in range(2):
                    nc.sync.dma_start(out=q_sb[:, k, :], in_=q_d[k, :, :])

                # g, h first (only need kv)
                for t in range(NI):
                    ps = pj.tile([CA, IT], f32, tag="fg")
                    nc.tensor.matmul(
                        ps, lhsT=wgt[:, 0, :], rhs=kv_sb[:, 0, ts(t, IT)],
                        start=True, stop=False,
                    )
                    nc.tensor.matmul(
                        ps, lhsT=wgt[:, 1, :], rhs=kv_sb[:, 1, ts(t, IT)],
                        start=False, stop=True,
                    )
                    nc.scalar.activation(
                        out=g[:, ts(t, IT)], in_=ps, func=AF.Identity, bias=bgs
                    )
                for j in range(NJ):
                    ph = pj.tile([P, C], f32, tag="h")
                    nc.tensor.matmul(
                        ph, lhsT=kv_sb[:, 0, ts(j, P)], rhs=wht[:, 0, :],
                        start=True, stop=False,
                    )
                    nc.tensor.matmul(
                        ph, lhsT=kv_sb[:, 1, ts(j, P)], rhs=wht[:, 1, :],
                        start=False, stop=True,
                    )
                    nc.scalar.copy(out=h[:, j, :], in_=ph)
                for t in range(NI):
                    ps = pj.tile([CA, IT], f32, tag="fg")
                    nc.tensor.matmul(
                        ps, lhsT=wft[:, 0, :], rhs=q_sb[:, 0, ts(t, IT)],
                        start=True, stop=False,
                    )
                    nc.tensor.matmul(
                        ps, lhsT=wft[:, 1, :], rhs=q_sb[:, 1, ts(t, IT)],
                        start=False, stop=True,
                    )
                    nc.scalar.activation(
                        out=fT[:, ts(t, IT)], in_=ps, func=AF.Identity, bias=bfs
                    )

            # ---- main attention loop ----
            with (
                tc.tile_pool(name="stps", bufs=2, space="PSUM") as stp,
                tc.tile_pool(name="ops", bufs=2, space="PSUM") as op,
                tc.tile_pool(name="dps", bufs=2, space="PSUM") as dp,
                tc.tile_pool(name="epool", bufs=4) as ep,
                tc.tile_pool(name="spool", bufs=2) as sp,
                tc.tile_pool(name="outp", bufs=2) as outp,
            ):

                def st_exp(t, j):
                    st = stp.tile([P, IT], f32, tag="st")
                    nc.tensor.matmul(
                        st, lhsT=g[:, ts(j, P)], rhs=fT[:, ts(t, IT)],
                        start=True, stop=True,
                    )
                    E = ep.tile([P, IT], bf16)
                    nc.scalar.activation(out=E, in_=st, func=AF.Exp)
                    return E

                def o_group(j, E, o0, o1, dd):
                    first, last = j == 0, j == NJ - 1
                    nc.tensor.matmul(
                        o0, lhsT=h[:, j, 0:P], rhs=E, start=first, stop=last
                    )
                    nc.tensor.matmul(
                        o1, lhsT=h[:, j, P:C], rhs=E, start=first, stop=last
                    )
                    nc.tensor.matmul(
                        dd, lhsT=ones_col, rhs=E, start=first, stop=last
                    )

                def epilogue(o0, o1, dd, t):
                    s1 = sp.tile([1, IT], f32, tag="s1")
                    nc.vector.reciprocal(s1, dd[0:1, :])
                    s2 = sp.tile([1, IT], f32, tag="s2")
                    nc.vector.tensor_scalar_mul(s2, s1, gam[0:1, :])
                    bc = stp.tile([P, IT], f32, tag="st")
                    nc.tensor.matmul(bc, lhsT=ones_row, rhs=s2, start=True, stop=True)
                    bs = sp.tile([P, IT], f32, tag="bs")
                    nc.scalar.copy(out=bs, in_=bc)
                    for k, ok in enumerate((o0, o1)):
                        t1 = outp.tile([P, IT], f32, tag=f"out{k}")
                        nc.vector.tensor_mul(t1, ok, bs)
                        nc.vector.tensor_add(t1, t1, kvgb[:, k, ts(t, IT)])
                        nc.sync.dma_start(out=out_d[k, :, ts(t, IT)], in_=t1)

                pending = None
                for t in range(NI):
                    o0 = op.tile([P, IT], f32, tag="o0")
                    o1 = op.tile([P, IT], f32, tag="o1")
                    dd = dp.tile([1, IT], f32)
                    prev_E = st_exp(t, 0)
                    for j in range(NJ):
                        next_E = st_exp(t, j + 1) if j + 1 < NJ else None
                        o_group(j, prev_E, o0, o1, dd)
                        prev_E = next_E
                        if j == 1 and pending is not None:
                            epilogue(*pending)
                            pending = None
                    pending = (o0, o1, dd, t)
                epilogue(*pending)

    _split_multi_waits(nc)
    return nc


def _get_nc():
    if "nc" not in _cache:
        _cache["nc"] = _build()
    return _cache["nc"]


def kernel(
    query_input, key_value_input, Wf, bf, Wg, bg, Wh, bh, gamma
):
    from concourse.bass_utils import run_bass_kernel_spmd

    B = query_input.shape[0]
    assert B == NCORES

    nc = _get_nc()

    f32 = np.float32
    wft = np.ascontiguousarray(Wf.T.reshape(2, P, CA), dtype=f32)
    wgt = np.ascontiguousarray(Wg.T.reshape(2, P, CA), dtype=f32)
    wht = np.ascontiguousarray(Wh.T.reshape(2, P, C), dtype=f32)
    bfv = np.ascontiguousarray(bf.reshape(CA, 1), dtype=f32)
    bgv = np.ascontiguousarray(bg.reshape(CA, 1), dtype=f32)
    bhv = np.ascontiguousarray(bh.reshape(2, P).T, dtype=f32)
    gm = np.ascontiguousarray(gamma.reshape(1, 1), dtype=f32)

    in_maps = []
    for b in range(B):
        in_maps.append(
            {
                "q": np.ascontiguousarray(
                    query_input[b].reshape(2, P, N), dtype=f32
                ),
                "kv": np.ascontiguousarray(
                    key_value_input[b].reshape(2, P, N), dtype=f32
                ),
                "wft": wft,
                "wgt": wgt,
                "wht": wht,
                "bfv": bfv,
                "bgv": bgv,
                "bhv": bhv,
                "gam": gm,
            }
        )

    res = run_bass_kernel_spmd(nc, in_maps, core_ids=list(range(NCORES)))
    _cache["last_result"] = res
    out = np.empty((B, C, 64, 64), dtype=f32)
    for b in range(B):
        out[b] = res.results[b]["out"].reshape(C, 64, 64)
    return out


if __name__ == "__main__":
    rng = np.random.default_rng(0)
    inputs = {
        "query_input": rng.standard_normal((8, 256, 64, 64), dtype=np.float32),
        "key_value_input": rng.standard_normal((8, 256, 64, 64), dtype=np.float32),
        "Wf": rng.standard_normal((CA, C), dtype=np.float32) * 0.06,
        "bf": rng.standard_normal((CA,), dtype=np.float32) * 0.06,
        "Wg": rng.standard_normal((CA, C), dtype=np.float32) * 0.06,
        "bg": rng.standard_normal((CA,), dtype=np.float32) * 0.06,
        "Wh": rng.standard_normal((C, C), dtype=np.float32) * 0.06,
        "bh": rng.standard_normal((C,), dtype=np.float32) * 0.06,
        "gamma": np.zeros((1,), dtype=np.float32),
    }
    out = kernel(**inputs)
    print(out.shape, out.dtype)


# revision 10
# speedup vs baseline: 1.0006x; 1.0006x over previous
"""Trainium2 Bass kernel for SAGAN-style 2D self-attention (nn_Attention2d).

Reference computation (per batch element b):
    q  = query_input[b].reshape(Cq, N)          # N = H*W = 4096, Cq = 256
    kv = key_value_input[b].reshape(C, N)       # C = 256
    fT = Wf @ q + bf        # [32, N]   (f transposed)
    g  = Wg @ kv + bg       # [32, N]
    h  = (Wh @ kv + bh).T   # [N, C]
    beta = softmax(fT.T @ g, axis=-1)           # [N, N]
    o  = beta @ h                               # [N, C]
    out[b] = gamma * o.T + kv                   # [C, N] -> [C, H, W]

Sharding: data-parallel over batch, one batch element per NeuronCore (B=8,
8 cores, no collectives).

Per-core algorithm (all layouts transposed so softmax reductions ride the
matmul path; no on-chip transposes needed):
  - projections: fT [32,N], g [32,N] (token-major), h [N,C] (token, channel)
  - loop over i-tiles (512 query tokens):
      for each j-tile (128 key tokens):
        ST  = g_j^T @ fT_i          # [128 j, 512 i] logits, PSUM
        E   = exp(ST)               # ACT, bf16 -> SBUF (no max subtraction:
                                    #  logits are bounded ~ +-13 for this op)
        o0 += h_j[:, 0:128]^T @ E   # accumulate over j in PSUM  [128 c, 512 i]
        o1 += h_j[:,128:256]^T @ E
        d  += ones^T @ E            # softmax denominator row [1, 512 i]
      s  = gamma / d                # [1, 512]
      bs = ones_col^T @ s           # PE broadcast across partitions
      out_c = o_c * bs + (kv + gamma*bh)   # DVE, then DMA out
"""

import os
import numpy as np

P = 128          # partitions
N = 4096         # tokens (H*W)
CA = 32          # attention channels
C = 256          # kv channels
IT = 512         # i-tile (query tokens per tile)
NI = N // IT     # 8
NJ = N // P      # 32
NCORES = 8

_cache = {}


def _split_multi_waits(nc, keep=1):
    """This walrus build encodes at most one sem wait per instruction
    (setupSyncWait: 'Too many sync wait commands').  Tile's sem assignment
    can attach several.  Move excess waits onto single-wait NoOps emitted
    just before the instruction on the same engine (engines execute their
    stream in order, so the waits still gate the instruction)."""
    import concourse.mybir as mybir
    import bass_rust

    for fn in nc.m.functions:
        for blk in fn.blocks:
            out = []
            for inst in blk.instructions:
                si = inst.sync_info
                if si is not None and len(si.on_wait) > keep:
                    waits = list(si.on_wait)
                    for k, w in enumerate(waits[:-keep]):
                        nop = mybir.InstNoOp(
                            name=f"{inst.name}_prewait{k}", ins=[], outs=[]
                        )
                        nop.engine = inst.engine
                        nop.sync_info = bass_rust.SyncInfo(on_wait=[w], on_update=[])
                        out.append(nop)
                    inst.sync_info = bass_rust.SyncInfo(
                        on_wait=waits[-keep:], on_update=list(si.on_update)
                    )
                out.append(inst)
            blk.instructions = out


def _build():
    import concourse.bass as bass
    import concourse.mybir as mybir
    from concourse.tile import TileContext
    from concourse.bass import ts

    f32 = mybir.dt.float32
    bf16 = mybir.dt.bfloat16
    AF = mybir.ActivationFunctionType

    nc = bass.Bass()
    q_d = nc.dram_tensor("q", [2, P, N], f32, kind="ExternalInput")
    kv_d = nc.dram_tensor("kv", [2, P, N], f32, kind="ExternalInput")
    wft_d = nc.dram_tensor("wft", [2, P, CA], f32, kind="ExternalInput")
    wgt_d = nc.dram_tensor("wgt", [2, P, CA], f32, kind="ExternalInput")
    wht_d = nc.dram_tensor("wht", [2, P, C], f32, kind="ExternalInput")
    bf_d = nc.dram_tensor("bfv", [CA, 1], f32, kind="ExternalInput")
    bg_d = nc.dram_tensor("bgv", [CA, 1], f32, kind="ExternalInput")
    bh_d = nc.dram_tensor("bhv", [P, 2], f32, kind="ExternalInput")
    gm_d = nc.dram_tensor("gam", [1, 1], f32, kind="ExternalInput")
    out_d = nc.dram_tensor("out", [2, P, N], f32, kind="ExternalOutput")

    with TileContext(nc) as tc:
        with (
            tc.tile_pool(name="const", bufs=1) as const,
            tc.tile_pool(name="big", bufs=1) as big,
        ):
            # ---- constants / small params ----
            wft = const.tile([P, 2, CA], f32)
            wgt = const.tile([P, 2, CA], f32)
            wht = const.tile([P, 2, C], f32)
            for k in range(2):
                nc.sync.dma_start(out=wft[:, k, :], in_=wft_d[k, :, :])
                nc.sync.dma_start(out=wgt[:, k, :], in_=wgt_d[k, :, :])
                nc.sync.dma_start(out=wht[:, k, :], in_=wht_d[k, :, :])
            bfs = const.tile([CA, 1], f32)
            bgs = const.tile([CA, 1], f32)
            nc.sync.dma_start(out=bfs, in_=bf_d[:, :])
            nc.sync.dma_start(out=bgs, in_=bg_d[:, :])
            bhs = const.tile([P, 2], f32)
            nc.sync.dma_start(out=bhs, in_=bh_d[:, :])
            gam = const.tile([P, 1], f32)
            nc.sync.dma_start(out=gam, in_=gm_d[:, :].to_broadcast([P, 1]))
            ones_col = const.tile([P, 1], bf16)
            nc.vector.memset(ones_col, 1.0)
            ones_row = const.tile([1, P], f32)
            nc.vector.memset(ones_row, 1.0)

            # gbh = gamma * bh  (per-partition, [128, 2])
            gbh = const.tile([P, 2], f32)
            nc.vector.tensor_scalar_mul(gbh, bhs, gam)

            # ---- big SBUF residents ----
            kv_sb = big.tile([P, 2, N], f32)
            for k in range(2):
                nc.sync.dma_start(out=kv_sb[:, k, :], in_=kv_d[k, :, :])
            kvgb = big.tile([P, 2, N], f32)  # kv + gamma*bh
            fT = big.tile([CA, N], bf16)
            g = big.tile([CA, N], bf16)
            h = big.tile([P, NJ, C], bf16)  # [token-in-jtile, jtile, channel]

            # kvgb = kv + gamma*bh (per-partition bias add on ACT)
            for k in range(2):
                nc.scalar.activation(
                    out=kvgb[:, k, :],
                    in_=kv_sb[:, k, :],
                    func=AF.Identity,
                    bias=gbh[:, k : k + 1],
                )

            # ---- projections ----
            with (
                tc.tile_pool(name="qpool", bufs=1) as qpool,
                tc.tile_pool(name="projps", bufs=2, space="PSUM") as pj,
            ):
                q_sb = qpool.tile([P, 2, N], f32)
                for k in range(2):
                    nc.sync.dma_start(out=q_sb[:, k, :], in_=q_d[k, :, :])

                # g, h first (only need kv)
                for t in range(NI):
                    ps = pj.tile([CA, IT], f32, tag="fg")
                    nc.tensor.matmul(
                        ps, lhsT=wgt[:, 0, :], rhs=kv_sb[:, 0, ts(t, IT)],
                        start=True, stop=False,
                    )
                    nc.tensor.matmul(
                        ps, lhsT=wgt[:, 1, :], rhs=kv_sb[:, 1, ts(t, IT)],
                        start=False, stop=True,
                    )
                    nc.scalar.activation(
                        out=g[:, ts(t, IT)], in_=ps, func=AF.Identity, bias=bgs
                    )
                for j in range(NJ):
                    ph = pj.tile([P, C], f32, tag="h")
                    nc.tensor.matmul(
                        ph, lhsT=kv_sb[:, 0, ts(j, P)], rhs=wht[:, 0, :],
                        start=True, stop=False,
                    )
                    nc.tensor.matmul(
                        ph, lhsT=kv_sb[:, 1, ts(j, P)], rhs=wht[:, 1, :],
                        start=False, stop=True,
                    )
                    nc.vector.tensor_copy(out=h[:, j, :], in_=ph)
                for t in range(NI):
                    ps = pj.tile([CA, IT], f32, tag="fg")
                    nc.tensor.matmul(
                        ps, lhsT=wft[:, 0, :], rhs=q_sb[:, 0, ts(t, IT)],
                        start=True, stop=False,
                    )
                    nc.tensor.matmul(
                        ps, lhsT=wft[:, 1, :], rhs=q_sb[:, 1, ts(t, IT)],
                        start=False, stop=True,
                    )
                    nc.scalar.activation(
                        out=fT[:, ts(t, IT)], in_=ps, func=AF.Identity, bias=bfs
                    )

            # ---- main attention loop ----
            # PSUM banks: st bufs=2 (+bc shares tag) = 2, o0/o1 bufs=2 = 4,
            # d bufs=2 = 2  -> 8 total.
            with (
                tc.tile_pool(name="stps", bufs=2, space="PSUM") as stp,
                tc.tile_pool(name="ops", bufs=2, space="PSUM") as op,
                tc.tile_pool(name="dps", bufs=2, space="PSUM") as dp,
                tc.tile_pool(name="epool", bufs=4) as ep,
                tc.tile_pool(name="spool", bufs=2) as sp,
                tc.tile_pool(name="outp", bufs=2) as outp,
            ):

                def st_exp(t, j):
                    st = stp.tile([P, IT], f32, tag="st")
                    nc.tensor.matmul(
                        st, lhsT=g[:, ts(j, P)], rhs=fT[:, ts(t, IT)],
                        start=True, stop=True,
                    )
                    E = ep.tile([P, IT], bf16)
                    nc.scalar.activation(out=E, in_=st, func=AF.Exp)
                    return E

                def o_group(j, E, o0, o1, dd):
                    # three matmuls sharing the same rhs (E) back-to-back
                    first, last = j == 0, j == NJ - 1
                    nc.tensor.matmul(
                        o0, lhsT=h[:, j, 0:P], rhs=E, start=first, stop=last
                    )
                    nc.tensor.matmul(
                        o1, lhsT=h[:, j, P:C], rhs=E, start=first, stop=last
                    )
                    nc.tensor.matmul(
                        dd, lhsT=ones_col, rhs=E, start=first, stop=last
                    )

                def epilogue(o0, o1, dd, t):
                    # d row -> SBUF, broadcast across partitions via PE, then
                    # reciprocal at full lane parallelism.
                    d_sb = sp.tile([1, IT], f32, tag="dsb")
                    nc.vector.tensor_copy(out=d_sb, in_=dd[0:1, :])
                    bc = stp.tile([P, IT], f32, tag="st")
                    nc.tensor.matmul(
                        bc, lhsT=ones_row, rhs=d_sb, start=True, stop=True
                    )
                    rbc = sp.tile([P, IT], f32, tag="rbc")
                    nc.vector.reciprocal(rbc, bc)
                    for k, ok in enumerate((o0, o1)):
                        t1 = outp.tile([P, IT], f32, tag=f"out{k}")
                        # t1 = (o * gamma) * (1/d)
                        nc.vector.scalar_tensor_tensor(
                            out=t1, in0=ok, scalar=gam, in1=rbc,
                            op0=mybir.AluOpType.mult, op1=mybir.AluOpType.mult,
                        )
                        nc.vector.tensor_add(t1, t1, kvgb[:, k, ts(t, IT)])
                        nc.sync.dma_start(out=out_d[k, :, ts(t, IT)], in_=t1)

                # one-pair-deep software pipeline: ST/exp of pair k+1 are
                # issued to PE/ACT before the o-groups of pair k, so the exp
                # latency hides under the previous pair's o-matmuls.
                pairs = [
                    (t, 2 * jp, 2 * jp + 1)
                    for t in range(NI)
                    for jp in range(NJ // 2)
                ]
                cur = {}
                pendingE = None
                pending_ep = None

                def flush_pair(pe):
                    nonlocal pending_ep
                    pt, pja, pjb, pEa, pEb = pe
                    o0, o1, dd = cur[pt]
                    o_group(pja, pEa, o0, o1, dd)
                    if pending_ep is not None and pja == 4:
                        epilogue(*pending_ep)
                        pending_ep = None
                    o_group(pjb, pEb, o0, o1, dd)
                    if pjb == NJ - 1:
                        pending_ep = (o0, o1, dd, pt)
                        del cur[pt]

                for t, ja, jb in pairs:
                    if ja == 0:
                        cur[t] = (
                            op.tile([P, IT], f32, tag="o0", name=f"o0_{t}"),
                            op.tile([P, IT], f32, tag="o1", name=f"o1_{t}"),
                            dp.tile([1, IT], f32, tag="dd", name=f"dd_{t}"),
                        )
                    Ea = st_exp(t, ja)
                    Eb = st_exp(t, jb)
                    if pendingE is not None:
                        flush_pair(pendingE)
                    pendingE = (t, ja, jb, Ea, Eb)
                flush_pair(pendingE)
                epilogue(*pending_ep)

    _split_multi_waits(nc)
    return nc


def _get_nc():
    if "nc" not in _cache:
        _cache["nc"] = _build()
    return _cache["nc"]


def kernel(
    query_input, key_value_input, Wf, bf, Wg, bg, Wh, bh, gamma
):
    from concourse.bass_utils import run_bass_kernel_spmd

    B = query_input.shape[0]
    assert B == NCORES

    nc = _get_nc()

    f32 = np.float32
    wft = np.ascontiguousarray(Wf.T.reshape(2, P, CA), dtype=f32)
    wgt = np.ascontiguousarray(Wg.T.reshape(2, P, CA), dtype=f32)
    wht = np.ascontiguousarray(Wh.T.reshape(2, P, C), dtype=f32)
    bfv = np.ascontiguousarray(bf.reshape(CA, 1), dtype=f32)
    bgv = np.ascontiguousarray(bg.reshape(CA, 1), dtype=f32)
    bhv = np.ascontiguousarray(bh.reshape(2, P).T, dtype=f32)
    gm = np.ascontiguousarray(gamma.reshape(1, 1), dtype=f32)

    in_maps = []
    for b in range(B):
        in_maps.append(
            {
                "q": np.ascontiguousarray(
                    query_input[b].reshape(2, P, N), dtype=f32
                ),
                "kv": np.ascontiguousarray(
                    key_value_input[b].reshape(2, P, N), dtype=f32
                ),
                "wft": wft,
                "wgt": wgt,
                "wht": wht,
                "bfv": bfv,
                "bgv": bgv,
                "bhv": bhv,
                "gam": gm,
            }
        )

    res = run_bass_kernel_spmd(nc, in_maps, core_ids=list(range(NCORES)))
    _cache["last_result"] = res
    out = np.empty((B, C, 64, 64), dtype=f32)
    for b in range(B):
        out[b] = res.results[b]["out"].reshape(C, 64, 64)
    return out


if __name__ == "__main__":
    rng = np.random.default_rng(0)
    inputs = {
        "query_input": rng.standard_normal((8, 256, 64, 64), dtype=np.float32),
        "key_value_input": rng.standard_normal((8, 256, 64, 64), dtype=np.float32),
        "Wf": rng.standard_normal((CA, C), dtype=np.float32) * 0.06,
        "bf": rng.standard_normal((CA,), dtype=np.float32) * 0.06,
        "Wg": rng.standard_normal((CA, C), dtype=np.float32) * 0.06,
        "bg": rng.standard_normal((CA,), dtype=np.float32) * 0.06,
        "Wh": rng.standard_normal((C, C), dtype=np.float32) * 0.06,
        "bh": rng.standard_normal((C,), dtype=np.float32) * 0.06,
        "gamma": np.zeros((1,), dtype=np.float32),
    }
    out = kernel(**inputs)
    print(out.shape, out.dtype)


# revision 12
# speedup vs baseline: 1.2356x; 1.2348x over previous
"""Trainium2 Bass kernel for SAGAN-style 2D self-attention (nn_Attention2d).

Reference computation (per batch element b):
    q  = query_input[b].reshape(Cq, N)          # N = H*W = 4096, Cq = 256
    kv = key_value_input[b].reshape(C, N)       # C = 256
    fT = Wf @ q + bf        # [32, N]   (f transposed)
    g  = Wg @ kv + bg       # [32, N]
    h  = (Wh @ kv + bh).T   # [N, C]
    beta = softmax(fT.T @ g, axis=-1)           # [N, N]
    o  = beta @ h                               # [N, C]
    out[b] = gamma * o.T + kv                   # [C, N] -> [C, H, W]

Sharding: data-parallel over batch, one batch element per NeuronCore (B=8,
8 cores, no collectives).

Per-core algorithm (all layouts transposed so softmax reductions ride the
matmul path; no on-chip transposes needed):
  - projections: fT [32,N], g [32,N] (token-major), h [N,C] (token, channel)
  - loop over i-tiles (512 query tokens):
      for each j-tile (128 key tokens):
        ST  = g_j^T @ fT_i          # [128 j, 512 i] logits, PSUM
        E   = exp(ST)               # ACT, bf16 -> SBUF (no max subtraction:
                                    #  logits are bounded ~ +-13 for this op)
        o0 += h_j[:, 0:128]^T @ E   # accumulate over j in PSUM  [128 c, 512 i]
        o1 += h_j[:,128:256]^T @ E
        d  += ones^T @ E            # softmax denominator row [1, 512 i]
      s  = gamma / d                # [1, 512]
      bs = ones_col^T @ s           # PE broadcast across partitions
      out_c = o_c * bs + (kv + gamma*bh)   # DVE, then DMA out
"""

import os
import numpy as np

P = 128          # partitions
N = 4096         # tokens (H*W)
CA = 32          # attention channels
C = 256          # kv channels
IT = 512         # i-tile (query tokens per tile)
NI = N // IT     # 8
NJ = N // P      # 32
NCORES = 8

_cache = {}


def _split_multi_waits(nc, keep=1):
    """This walrus build encodes at most one sem wait per instruction
    (setupSyncWait: 'Too many sync wait commands').  Tile's sem assignment
    can attach several.  Move excess waits onto single-wait NoOps emitted
    just before the instruction on the same engine (engines execute their
    stream in order, so the waits still gate the instruction)."""
    import concourse.mybir as mybir
    import bass_rust

    for fn in nc.m.functions:
        for blk in fn.blocks:
            out = []
            for inst in blk.instructions:
                si = inst.sync_info
                if si is not None and len(si.on_wait) > keep:
                    waits = list(si.on_wait)
                    for k, w in enumerate(waits[:-keep]):
                        nop = mybir.InstNoOp(
                            name=f"{inst.name}_prewait{k}", ins=[], outs=[]
                        )
                        nop.engine = inst.engine
                        nop.sync_info = bass_rust.SyncInfo(on_wait=[w], on_update=[])
                        out.append(nop)
                    inst.sync_info = bass_rust.SyncInfo(
                        on_wait=waits[-keep:], on_update=list(si.on_update)
                    )
                out.append(inst)
            blk.instructions = out


def _build():
    import concourse.bass as bass
    import concourse.mybir as mybir
    from concourse.tile import TileContext
    from concourse.bass import ts

    f32 = mybir.dt.float32
    bf16 = mybir.dt.bfloat16
    AF = mybir.ActivationFunctionType

    nc = bass.Bass()
    q_d = nc.dram_tensor("q", [2, P, N], f32, kind="ExternalInput")
    kv_d = nc.dram_tensor("kv", [2, P, N], f32, kind="ExternalInput")
    wft_d = nc.dram_tensor("wft", [2, P, CA], f32, kind="ExternalInput")
    wgt_d = nc.dram_tensor("wgt", [2, P, CA], f32, kind="ExternalInput")
    wht_d = nc.dram_tensor("wht", [2, P, C], f32, kind="ExternalInput")
    bf_d = nc.dram_tensor("bfv", [CA, 1], f32, kind="ExternalInput")
    bg_d = nc.dram_tensor("bgv", [CA, 1], f32, kind="ExternalInput")
    bh_d = nc.dram_tensor("bhv", [P, 2], f32, kind="ExternalInput")
    gm_d = nc.dram_tensor("gam", [1, 1], f32, kind="ExternalInput")
    out_d = nc.dram_tensor("out", [2, P, N], f32, kind="ExternalOutput")

    with TileContext(nc) as tc:
        with (
            tc.tile_pool(name="const", bufs=1) as const,
            tc.tile_pool(name="big", bufs=1) as big,
        ):
            # ---- constants / small params ----
            wft = const.tile([P, 2, CA], f32)
            wgt = const.tile([P, 2, CA], f32)
            wht = const.tile([P, 2, C], f32)
            for k in range(2):
                nc.sync.dma_start(out=wft[:, k, :], in_=wft_d[k, :, :])
                nc.sync.dma_start(out=wgt[:, k, :], in_=wgt_d[k, :, :])
                nc.sync.dma_start(out=wht[:, k, :], in_=wht_d[k, :, :])
            bfs = const.tile([CA, 1], f32)
            bgs = const.tile([CA, 1], f32)
            nc.sync.dma_start(out=bfs, in_=bf_d[:, :])
            nc.sync.dma_start(out=bgs, in_=bg_d[:, :])
            bhs = const.tile([P, 2], f32)
            nc.sync.dma_start(out=bhs, in_=bh_d[:, :])
            gam = const.tile([P, 1], f32)
            nc.sync.dma_start(out=gam, in_=gm_d[:, :].to_broadcast([P, 1]))
            ones_col = const.tile([P, 1], bf16)
            nc.vector.memset(ones_col, 1.0)
            ones_row = const.tile([1, P], f32)
            nc.vector.memset(ones_row, 1.0)

            # gbh = gamma * bh  (per-partition, [128, 2])
            gbh = const.tile([P, 2], f32)
            nc.vector.tensor_scalar_mul(gbh, bhs, gam)

            # ---- big SBUF residents ----
            kv_sb = big.tile([P, 2, N], f32)
            for k in range(2):
                nc.sync.dma_start(out=kv_sb[:, k, :], in_=kv_d[k, :, :])
            kvgb = big.tile([P, 2, N], f32)  # kv + gamma*bh
            fT = big.tile([CA, N], bf16)
            g = big.tile([CA, N], bf16)
            h = big.tile([P, NJ, C], bf16)  # [token-in-jtile, jtile, channel]

            # kvgb = kv + gamma*bh (per-partition bias add on ACT)
            for k in range(2):
                nc.scalar.activation(
                    out=kvgb[:, k, :],
                    in_=kv_sb[:, k, :],
                    func=AF.Identity,
                    bias=gbh[:, k : k + 1],
                )

            # ---- projections ----
            with (
                tc.tile_pool(name="qpool", bufs=1) as qpool,
                tc.tile_pool(name="projps", bufs=2, space="PSUM") as pj,
            ):
                q_sb = qpool.tile([P, 2, N], f32)
                for k in range(2):
                    nc.sync.dma_start(out=q_sb[:, k, :], in_=q_d[k, :, :])

                # g, h first (only need kv)
                for t in range(NI):
                    ps = pj.tile([CA, IT], f32, tag="fg")
                    nc.tensor.matmul(
                        ps, lhsT=wgt[:, 0, :], rhs=kv_sb[:, 0, ts(t, IT)],
                        start=True, stop=False,
                    )
                    nc.tensor.matmul(
                        ps, lhsT=wgt[:, 1, :], rhs=kv_sb[:, 1, ts(t, IT)],
                        start=False, stop=True,
                    )
                    nc.scalar.activation(
                        out=g[:, ts(t, IT)], in_=ps, func=AF.Identity, bias=bgs
                    )
                for j in range(NJ):
                    ph = pj.tile([P, C], f32, tag="h")
                    nc.tensor.matmul(
                        ph, lhsT=kv_sb[:, 0, ts(j, P)], rhs=wht[:, 0, :],
                        start=True, stop=False,
                    )
                    nc.tensor.matmul(
                        ph, lhsT=kv_sb[:, 1, ts(j, P)], rhs=wht[:, 1, :],
                        start=False, stop=True,
                    )
                    nc.vector.tensor_copy(out=h[:, j, :], in_=ph)
                for t in range(NI):
                    ps = pj.tile([CA, IT], f32, tag="fg")
                    nc.tensor.matmul(
                        ps, lhsT=wft[:, 0, :], rhs=q_sb[:, 0, ts(t, IT)],
                        start=True, stop=False,
                    )
                    nc.tensor.matmul(
                        ps, lhsT=wft[:, 1, :], rhs=q_sb[:, 1, ts(t, IT)],
                        start=False, stop=True,
                    )
                    nc.scalar.activation(
                        out=fT[:, ts(t, IT)], in_=ps, func=AF.Identity, bias=bfs
                    )

            # ---- main attention loop ----
            # PSUM banks: st bufs=2, o0/o1 bufs=2 = 4, d 1, bc 1 -> 8 total.
            with (
                tc.tile_pool(name="stps", bufs=2, space="PSUM") as stp,
                tc.tile_pool(name="ops", bufs=2, space="PSUM") as op,
                tc.tile_pool(name="dps", bufs=1, space="PSUM") as dp,
                tc.tile_pool(name="bcps", bufs=1, space="PSUM") as bcp,
                tc.tile_pool(name="epool", bufs=6) as ep,
                tc.tile_pool(name="spool", bufs=2) as sp,
                tc.tile_pool(name="outp", bufs=2) as outp,
            ):

                def st_exp(t, j):
                    st = stp.tile([P, IT], f32, tag="st")
                    nc.tensor.matmul(
                        st, lhsT=g[:, ts(j, P)], rhs=fT[:, ts(t, IT)],
                        start=True, stop=True,
                    )
                    E = ep.tile([P, IT], bf16)
                    nc.scalar.activation(out=E, in_=st, func=AF.Exp)
                    return E

                def o_pair(ja, jb, Ea, Eb, o0, o1, dd):
                    # o-matmuls first (same-rhs runs), then both d-matmuls
                    # back-to-back so they share the stationary `ones` operand
                    # (no LDWEIGHTS between them).
                    fa, la = ja == 0, jb == NJ - 1
                    nc.tensor.matmul(o0, lhsT=h[:, ja, 0:P], rhs=Ea, start=fa, stop=False)
                    nc.tensor.matmul(o1, lhsT=h[:, ja, P:C], rhs=Ea, start=fa, stop=False)
                    nc.tensor.matmul(o0, lhsT=h[:, jb, 0:P], rhs=Eb, start=False, stop=la)
                    nc.tensor.matmul(o1, lhsT=h[:, jb, P:C], rhs=Eb, start=False, stop=la)
                    nc.tensor.matmul(dd, lhsT=ones_col, rhs=Ea, start=fa, stop=False)
                    nc.tensor.matmul(dd, lhsT=ones_col, rhs=Eb, start=False, stop=la)

                def epilogue(o0, o1, dd, t):
                    # d row -> SBUF, broadcast across partitions via PE, then
                    # reciprocal at full lane parallelism.
                    d_sb = sp.tile([1, IT], f32, tag="dsb")
                    nc.vector.tensor_copy(out=d_sb, in_=dd[0:1, :])
                    bc = bcp.tile([P, IT], f32, tag="bc")
                    nc.tensor.matmul(
                        bc, lhsT=ones_row, rhs=d_sb, start=True, stop=True
                    )
                    rbc = sp.tile([P, IT], f32, tag="rbc")
                    nc.vector.reciprocal(rbc, bc)
                    for k, ok in enumerate((o0, o1)):
                        t1 = outp.tile([P, IT], f32, tag=f"out{k}")
                        # t1 = (o * gamma) * (1/d)
                        nc.vector.scalar_tensor_tensor(
                            out=t1, in0=ok, scalar=gam, in1=rbc,
                            op0=mybir.AluOpType.mult, op1=mybir.AluOpType.mult,
                        )
                        nc.vector.tensor_add(t1, t1, kvgb[:, k, ts(t, IT)])
                        nc.sync.dma_start(out=out_d[k, :, ts(t, IT)], in_=t1)

                # one-pair-deep software pipeline: ST/exp of pair k+1 are
                # issued to PE/ACT before the o-groups of pair k, so the exp
                # latency hides under the previous pair's o-matmuls.
                pairs = [
                    (t, 2 * jp, 2 * jp + 1)
                    for t in range(NI)
                    for jp in range(NJ // 2)
                ]
                cur = {}
                pendingE = None
                pending_ep = None

                def flush_pair(pe):
                    nonlocal pending_ep
                    pt, pja, pjb, pEa, pEb = pe
                    o0, o1, dd = cur[pt]
                    o_pair(pja, pjb, pEa, pEb, o0, o1, dd)
                    if pending_ep is not None and pja == 4:
                        epilogue(*pending_ep)
                        pending_ep = None
                    if pjb == NJ - 1:
                        pending_ep = (o0, o1, dd, pt)
                        del cur[pt]

                for t, ja, jb in pairs:
                    if ja == 0:
                        cur[t] = (
                            op.tile([P, IT], f32, tag="o0", name=f"o0_{t}"),
                            op.tile([P, IT], f32, tag="o1", name=f"o1_{t}"),
                            dp.tile([1, IT], f32, tag="dd", name=f"dd_{t}"),
                        )
                    Ea = st_exp(t, ja)
                    Eb = st_exp(t, jb)
                    if pendingE is not None:
                        flush_pair(pendingE)
                    pendingE = (t, ja, jb, Ea, Eb)
                flush_pair(pendingE)
                epilogue(*pending_ep)

    _split_multi_waits(nc)
    return nc


def _get_nc():
    if "nc" not in _cache:
        _cache["nc"] = _build()
    return _cache["nc"]


def kernel(
    query_input, key_value_input, Wf, bf, Wg, bg, Wh, bh, gamma
):
    from concourse.bass_utils import run_bass_kernel_spmd

    B = query_input.shape[0]
    assert B == NCORES

    nc = _get_nc()

    f32 = np.float32
    wft = np.ascontiguousarray(Wf.T.reshape(2, P, CA), dtype=f32)
    wgt = np.ascontiguousarray(Wg.T.reshape(2, P, CA), dtype=f32)
    wht = np.ascontiguousarray(Wh.T.reshape(2, P, C), dtype=f32)
    bfv = np.ascontiguousarray(bf.reshape(CA, 1), dtype=f32)
    bgv = np.ascontiguousarray(bg.reshape(CA, 1), dtype=f32)
    bhv = np.ascontiguousarray(bh.reshape(2, P).T, dtype=f32)
    gm = np.ascontiguousarray(gamma.reshape(1, 1), dtype=f32)

    in_maps = []
    for b in range(B):
        in_maps.append(
            {
                "q": np.ascontiguousarray(
                    query_input[b].reshape(2, P, N), dtype=f32
                ),
                "kv": np.ascontiguousarray(
                    key_value_input[b].reshape(2, P, N), dtype=f32
                ),
                "wft": wft,
                "wgt": wgt,
                "wht": wht,
                "bfv": bfv,
                "bgv": bgv,
                "bhv": bhv,
                "gam": gm,
            }
        )

    res = run_bass_kernel_spmd(nc, in_maps, core_ids=list(range(NCORES)))
    _cache["last_result"] = res
    out = np.empty((B, C, 64, 64), dtype=f32)
    for b in range(B):
        out[b] = res.results[b]["out"].reshape(C, 64, 64)
    return out


if __name__ == "__main__":
    rng = np.random.default_rng(0)
    inputs = {
        "query_input": rng.standard_normal((8, 256, 64, 64), dtype=np.float32),
        "key_value_input": rng.standard_normal((8, 256, 64, 64), dtype=np.float32),
        "Wf": rng.standard_normal((CA, C), dtype=np.float32) * 0.06,
        "bf": rng.standard_normal((CA,), dtype=np.float32) * 0.06,
        "Wg": rng.standard_normal((CA, C), dtype=np.float32) * 0.06,
        "bg": rng.standard_normal((CA,), dtype=np.float32) * 0.06,
        "Wh": rng.standard_normal((C, C), dtype=np.float32) * 0.06,
        "bh": rng.standard_normal((C,), dtype=np.float32) * 0.06,
        "gamma": np.zeros((1,), dtype=np.float32),
    }
    out = kernel(**inputs)
    print(out.shape, out.dtype)


# revision 14
# speedup vs baseline: 1.4833x; 1.2005x over previous
"""Trainium2 Bass kernel for SAGAN-style 2D self-attention (nn_Attention2d).

Reference computation (per batch element b):
    q  = query_input[b].reshape(Cq, N)          # N = H*W = 4096, Cq = 256
    kv = key_value_input[b].reshape(C, N)       # C = 256
    fT = Wf @ q + bf        # [32, N]   (f transposed)
    g  = Wg @ kv + bg       # [32, N]
    h  = (Wh @ kv + bh).T   # [N, C]
    beta = softmax(fT.T @ g, axis=-1)           # [N, N]
    o  = beta @ h                               # [N, C]
    out[b] = gamma * o.T + kv                   # [C, N] -> [C, H, W]

Sharding: data-parallel over batch, one batch element per NeuronCore (B=8,
8 cores, no collectives).

Per-core algorithm (all layouts transposed so softmax reductions ride the
matmul path; no on-chip transposes needed):
  - projections: fT [32,N], g [32,N] (token-major), h [N,C] (token, channel)
  - loop over i-tiles (512 query tokens):
      for each j-tile (128 key tokens):
        ST  = g_j^T @ fT_i          # [128 j, 512 i] logits, PSUM
        E   = exp(ST)               # ACT, bf16 -> SBUF (no max subtraction:
                                    #  logits are bounded ~ +-13 for this op)
        o0 += h_j[:, 0:128]^T @ E   # accumulate over j in PSUM  [128 c, 512 i]
        o1 += h_j[:,128:256]^T @ E
        d  += ones^T @ E            # softmax denominator row [1, 512 i]
      s  = gamma / d                # [1, 512]
      bs = ones_col^T @ s           # PE broadcast across partitions
      out_c = o_c * bs + (kv + gamma*bh)   # DVE, then DMA out
"""

import os
import numpy as np

P = 128          # partitions
N = 4096         # tokens (H*W)
CA = 32          # attention channels
C = 256          # kv channels
IT = 512         # i-tile (query tokens per tile)
NI = N // IT     # 8
NJ = N // P      # 32
NCORES = 8

_cache = {}


def _split_multi_waits(nc, keep=1):
    """This walrus build encodes at most one sem wait per instruction
    (setupSyncWait: 'Too many sync wait commands').  Tile's sem assignment
    can attach several.  Move excess waits onto single-wait NoOps emitted
    just before the instruction on the same engine (engines execute their
    stream in order, so the waits still gate the instruction)."""
    import concourse.mybir as mybir
    import bass_rust

    for fn in nc.m.functions:
        for blk in fn.blocks:
            out = []
            for inst in blk.instructions:
                si = inst.sync_info
                if si is not None and len(si.on_wait) > keep:
                    waits = list(si.on_wait)
                    for k, w in enumerate(waits[:-keep]):
                        nop = mybir.InstNoOp(
                            name=f"{inst.name}_prewait{k}", ins=[], outs=[]
                        )
                        nop.engine = inst.engine
                        nop.sync_info = bass_rust.SyncInfo(on_wait=[w], on_update=[])
                        out.append(nop)
                    inst.sync_info = bass_rust.SyncInfo(
                        on_wait=waits[-keep:], on_update=list(si.on_update)
                    )
                out.append(inst)
            blk.instructions = out


def _build():
    import concourse.bass as bass
    import concourse.mybir as mybir
    from concourse.tile import TileContext
    from concourse.bass import ts

    f32 = mybir.dt.float32
    bf16 = mybir.dt.bfloat16
    AF = mybir.ActivationFunctionType

    nc = bass.Bass()
    q_d = nc.dram_tensor("q", [2, P, N], f32, kind="ExternalInput")
    kv_d = nc.dram_tensor("kv", [2, P, N], f32, kind="ExternalInput")
    wft_d = nc.dram_tensor("wft", [2, P, CA], f32, kind="ExternalInput")
    wgt_d = nc.dram_tensor("wgt", [2, P, CA], f32, kind="ExternalInput")
    wht_d = nc.dram_tensor("wht", [2, P, C], f32, kind="ExternalInput")
    bf_d = nc.dram_tensor("bfv", [CA, 1], f32, kind="ExternalInput")
    bg_d = nc.dram_tensor("bgv", [CA, 1], f32, kind="ExternalInput")
    bh_d = nc.dram_tensor("bhv", [P, 2], f32, kind="ExternalInput")
    gm_d = nc.dram_tensor("gam", [1, 1], f32, kind="ExternalInput")
    out_d = nc.dram_tensor("out", [2, P, N], f32, kind="ExternalOutput")

    with TileContext(nc) as tc:
        with (
            tc.tile_pool(name="const", bufs=1) as const,
            tc.tile_pool(name="big", bufs=1) as big,
        ):
            # ---- constants / small params ----
            wft = const.tile([P, 2, CA], f32)
            wgt = const.tile([P, 2, CA], f32)
            wht = const.tile([P, 2, C], f32)
            for k in range(2):
                nc.sync.dma_start(out=wft[:, k, :], in_=wft_d[k, :, :])
                nc.sync.dma_start(out=wgt[:, k, :], in_=wgt_d[k, :, :])
                nc.sync.dma_start(out=wht[:, k, :], in_=wht_d[k, :, :])
            bfs = const.tile([CA, 1], f32)
            bgs = const.tile([CA, 1], f32)
            nc.sync.dma_start(out=bfs, in_=bf_d[:, :])
            nc.sync.dma_start(out=bgs, in_=bg_d[:, :])
            bhs = const.tile([P, 2], f32)
            nc.sync.dma_start(out=bhs, in_=bh_d[:, :])
            gam = const.tile([P, 1], f32)
            nc.sync.dma_start(out=gam, in_=gm_d[:, :].to_broadcast([P, 1]))
            ones_col = const.tile([P, 1], bf16)
            nc.vector.memset(ones_col, 1.0)
            ones_row = const.tile([1, P], f32)
            nc.vector.memset(ones_row, 1.0)

            # gbh = gamma * bh  (per-partition, [128, 2])
            gbh = const.tile([P, 2], f32)
            nc.vector.tensor_scalar_mul(gbh, bhs, gam)

            # ---- big SBUF residents ----
            kv_sb = big.tile([P, 2, N], f32)
            for k in range(2):
                nc.sync.dma_start(out=kv_sb[:, k, :], in_=kv_d[k, :, :])
            kvgb = big.tile([P, 2, N], f32)  # kv + gamma*bh
            # fT4: fT replicated into 4 row strips (partitions 32s..32s+31)
            # so 4 ST matmuls (K=32) can run concurrently via row tiling.
            # g4: strip s, block k holds g j-tile (4k+s).
            fT4 = big.tile([P, N], bf16)
            g4 = big.tile([P, (NJ // 4) * P], bf16)
            g = big.tile([CA, N], bf16)
            h = big.tile([P, NJ, C], bf16)  # [token-in-jtile, jtile, channel]
            fT = fT4[0:CA, :]

            # kvgb = kv + gamma*bh (per-partition bias add on ACT)
            for k in range(2):
                nc.scalar.activation(
                    out=kvgb[:, k, :],
                    in_=kv_sb[:, k, :],
                    func=AF.Identity,
                    bias=gbh[:, k : k + 1],
                )

            # ---- projections ----
            with (
                tc.tile_pool(name="qpool", bufs=1) as qpool,
                tc.tile_pool(name="projps", bufs=2, space="PSUM") as pj,
            ):
                q_sb = qpool.tile([P, 2, N], f32)
                for k in range(2):
                    nc.sync.dma_start(out=q_sb[:, k, :], in_=q_d[k, :, :])

                # g, h first (only need kv)
                for t in range(NI):
                    ps = pj.tile([CA, IT], f32, tag="fg")
                    nc.tensor.matmul(
                        ps, lhsT=wgt[:, 0, :], rhs=kv_sb[:, 0, ts(t, IT)],
                        start=True, stop=False,
                    )
                    nc.tensor.matmul(
                        ps, lhsT=wgt[:, 1, :], rhs=kv_sb[:, 1, ts(t, IT)],
                        start=False, stop=True,
                    )
                    nc.scalar.activation(
                        out=g[:, ts(t, IT)], in_=ps, func=AF.Identity, bias=bgs
                    )
                for j in range(NJ):
                    ph = pj.tile([P, C], f32, tag="h")
                    nc.tensor.matmul(
                        ph, lhsT=kv_sb[:, 0, ts(j, P)], rhs=wht[:, 0, :],
                        start=True, stop=False,
                    )
                    nc.tensor.matmul(
                        ph, lhsT=kv_sb[:, 1, ts(j, P)], rhs=wht[:, 1, :],
                        start=False, stop=True,
                    )
                    nc.vector.tensor_copy(out=h[:, j, :], in_=ph)
                for t in range(NI):
                    ps = pj.tile([CA, IT], f32, tag="fg")
                    nc.tensor.matmul(
                        ps, lhsT=wft[:, 0, :], rhs=q_sb[:, 0, ts(t, IT)],
                        start=True, stop=False,
                    )
                    nc.tensor.matmul(
                        ps, lhsT=wft[:, 1, :], rhs=q_sb[:, 1, ts(t, IT)],
                        start=False, stop=True,
                    )
                    nc.scalar.activation(
                        out=fT[:, ts(t, IT)], in_=ps, func=AF.Identity, bias=bfs
                    )

                # replicate fT into row strips 1..3 and scatter g into g4
                for s in range(1, 4):
                    nc.sync.dma_start(
                        out=fT4[CA * s : CA * (s + 1), :], in_=fT
                    )
                g_blk = g.rearrange("p (k four c) -> p k four c", four=4, c=P)
                g4_blk = g4.rearrange("p (k c) -> p k c", c=P)
                for s in range(4):
                    nc.sync.dma_start(
                        out=g4_blk[CA * s : CA * (s + 1), :, :],
                        in_=g_blk[:, :, s, :],
                    )

            # ---- main attention loop ----
            # j-tiles processed in packs of 4: the 4 ST matmuls (K=32) run
            # concurrently in the PE's four 32-row groups, writing 4 PSUM
            # banks of one [128, 4*512] tile; one ACTIVATE exps all 2048
            # columns.  PSUM banks: st4 4, o0/o1 bufs=1 = 2, d 1, bc 1 -> 8.
            with (
                tc.tile_pool(name="stps", bufs=1, space="PSUM") as stp,
                tc.tile_pool(name="ops", bufs=1, space="PSUM") as op,
                tc.tile_pool(name="dps", bufs=1, space="PSUM") as dp,
                tc.tile_pool(name="bcps", bufs=1, space="PSUM") as bcp,
                tc.tile_pool(name="epool", bufs=3) as ep,
                tc.tile_pool(name="spool", bufs=2) as sp,
                tc.tile_pool(name="outp", bufs=2) as outp,
            ):
                NP4 = NJ // 4  # packs per i-tile

                def st_exp_pack(t, k):
                    st4 = stp.tile([P, 4, IT], f32, tag="st4", name=f"st4_{t}_{k}")
                    for s in range(4):
                        nc.tensor.matmul(
                            st4[:, s, :],
                            lhsT=g4[CA * s : CA * (s + 1), ts(k, P)],
                            rhs=fT4[CA * s : CA * (s + 1), ts(t, IT)],
                            start=True, stop=True,
                            tile_position=(CA * s, 0),
                        )
                    E4 = ep.tile([P, 4, IT], bf16, tag="e4", name=f"e4_{t}_{k}")
                    nc.scalar.activation(out=E4, in_=st4, func=AF.Exp)
                    return E4

                def flush_pack(t, k, E4, o0, o1, dd):
                    first, last = k == 0, k == NP4 - 1
                    for s in range(4):
                        j = 4 * k + s
                        nc.tensor.matmul(
                            o0, lhsT=h[:, j, 0:P], rhs=E4[:, s, :],
                            start=first and s == 0, stop=last and s == 3,
                        )
                        nc.tensor.matmul(
                            o1, lhsT=h[:, j, P:C], rhs=E4[:, s, :],
                            start=first and s == 0, stop=last and s == 3,
                        )
                    # d-matmuls last, back-to-back: they share the stationary
                    # `ones` operand so only the first needs a weight load.
                    for s in range(4):
                        nc.tensor.matmul(
                            dd, lhsT=ones_col, rhs=E4[:, s, :],
                            start=first and s == 0, stop=last and s == 3,
                        )

                def epilogue(o0, o1, dd, t):
                    d_sb = sp.tile([1, IT], f32, tag="dsb")
                    nc.vector.tensor_copy(out=d_sb, in_=dd[0:1, :])
                    bc = bcp.tile([P, IT], f32, tag="bc")
                    nc.tensor.matmul(
                        bc, lhsT=ones_row, rhs=d_sb, start=True, stop=True
                    )
                    t1s = []
                    for k, ok in enumerate((o0, o1)):
                        t1 = outp.tile(
                            [P, IT], f32, tag=f"out{k}", name=f"t1_{t}_{k}"
                        )
                        # release the o psum bank promptly (gamma scale only)
                        nc.vector.tensor_scalar_mul(t1, ok, gam)
                        t1s.append(t1)
                    rbc = sp.tile([P, IT], f32, tag="rbc")
                    nc.vector.reciprocal(rbc, bc)
                    for k, t1 in enumerate(t1s):
                        nc.vector.scalar_tensor_tensor(
                            out=t1, in0=t1, scalar=1.0, in1=rbc,
                            op0=mybir.AluOpType.mult, op1=mybir.AluOpType.mult,
                        )
                        nc.vector.tensor_add(t1, t1, kvgb[:, k, ts(t, IT)])
                        nc.sync.dma_start(out=out_d[k, :, ts(t, IT)], in_=t1)

                packs = [(t, k) for t in range(NI) for k in range(NP4)]
                cur = {}
                pendingE = None
                pending_ep = None
                for t, k in packs:
                    if k == 0:
                        cur[t] = (
                            op.tile([P, IT], f32, tag="o0", name=f"o0_{t}"),
                            op.tile([P, IT], f32, tag="o1", name=f"o1_{t}"),
                            dp.tile([1, IT], f32, tag="dd", name=f"dd_{t}"),
                        )
                    E4 = st_exp_pack(t, k)
                    if pendingE is not None:
                        pt, pk, pE4 = pendingE
                        o0, o1, dd = cur[pt]
                        flush_pack(pt, pk, pE4, o0, o1, dd)
                        if pending_ep is not None and pk == 1:
                            epilogue(*pending_ep)
                            pending_ep = None
                        if pk == NP4 - 1:
                            pending_ep = (o0, o1, dd, pt)
                            del cur[pt]
                    pendingE = (t, k, E4)
                pt, pk, pE4 = pendingE
                o0, o1, dd = cur[pt]
                flush_pack(pt, pk, pE4, o0, o1, dd)
                if pending_ep is not None:
                    epilogue(*pending_ep)
                epilogue(o0, o1, dd, pt)

    _split_multi_waits(nc)
    return nc


def _get_nc():
    if "nc" not in _cache:
        _cache["nc"] = _build()
    return _cache["nc"]


def kernel(
    query_input, key_value_input, Wf, bf, Wg, bg, Wh, bh, gamma
):
    from concourse.bass_utils import run_bass_kernel_spmd

    B = query_input.shape[0]
    assert B == NCORES

    nc = _get_nc()

    f32 = np.float32
    wft = np.ascontiguousarray(Wf.T.reshape(2, P, CA), dtype=f32)
    wgt = np.ascontiguousarray(Wg.T.reshape(2, P, CA), dtype=f32)
    wht = np.ascontiguousarray(Wh.T.reshape(2, P, C), dtype=f32)
    bfv = np.ascontiguousarray(bf.reshape(CA, 1), dtype=f32)
    bgv = np.ascontiguousarray(bg.reshape(CA, 1), dtype=f32)
    bhv = np.ascontiguousarray(bh.reshape(2, P).T, dtype=f32)
    gm = np.ascontiguousarray(gamma.reshape(1, 1), dtype=f32)

    in_maps = []
    for b in range(B):
        in_maps.append(
            {
                "q": np.ascontiguousarray(
                    query_input[b].reshape(2, P, N), dtype=f32
                ),
                "kv": np.ascontiguousarray(
                    key_value_input[b].reshape(2, P, N), dtype=f32
                ),
                "wft": wft,
                "wgt": wgt,
                "wht": wht,
                "bfv": bfv,
                "bgv": bgv,
                "bhv": bhv,
                "gam": gm,
            }
        )

    res = run_bass_kernel_spmd(nc, in_maps, core_ids=list(range(NCORES)))
    _cache["last_result"] = res
    out = np.empty((B, C, 64, 64), dtype=f32)
    for b in range(B):
        out[b] = res.results[b]["out"].reshape(C, 64, 64)
    return out


if __name__ == "__main__":
    rng = np.random.default_rng(0)
    inputs = {
        "query_input": rng.standard_normal((8, 256, 64, 64), dtype=np.float32),
        "key_value_input": rng.standard_normal((8, 256, 64, 64), dtype=np.float32),
        "Wf": rng.standard_normal((CA, C), dtype=np.float32) * 0.06,
        "bf": rng.standard_normal((CA,), dtype=np.float32) * 0.06,
        "Wg": rng.standard_normal((CA, C), dtype=np.float32) * 0.06,
        "bg": rng.standard_normal((CA,), dtype=np.float32) * 0.06,
        "Wh": rng.standard_normal((C, C), dtype=np.float32) * 0.06,
        "bh": rng.standard_normal((C,), dtype=np.float32) * 0.06,
        "gamma": np.zeros((1,), dtype=np.float32),
    }
    out = kernel(**inputs)
    print(out.shape, out.dtype)


# revision 15
# speedup vs baseline: 1.6546x; 1.1155x over previous
"""Trainium2 Bass kernel for SAGAN-style 2D self-attention (nn_Attention2d).

Reference computation (per batch element b):
    q  = query_input[b].reshape(Cq, N)          # N = H*W = 4096, Cq = 256
    kv = key_value_input[b].reshape(C, N)       # C = 256
    fT = Wf @ q + bf        # [32, N]   (f transposed)
    g  = Wg @ kv + bg       # [32, N]
    h  = (Wh @ kv + bh).T   # [N, C]
    beta = softmax(fT.T @ g, axis=-1)           # [N, N]
    o  = beta @ h                               # [N, C]
    out[b] = gamma * o.T + kv                   # [C, N] -> [C, H, W]

Sharding: data-parallel over batch, one batch element per NeuronCore (B=8,
8 cores, no collectives).

Per-core algorithm (all layouts transposed so softmax reductions ride the
matmul path; no on-chip transposes needed):
  - projections: fT [32,N], g [32,N] (token-major), h [N,C] (token, channel)
  - loop over i-tiles (512 query tokens):
      for each j-tile (128 key tokens):
        ST  = g_j^T @ fT_i          # [128 j, 512 i] logits, PSUM
        E   = exp(ST)               # ACT, bf16 -> SBUF (no max subtraction:
                                    #  logits are bounded ~ +-13 for this op)
        o0 += h_j[:, 0:128]^T @ E   # accumulate over j in PSUM  [128 c, 512 i]
        o1 += h_j[:,128:256]^T @ E
        d  += ones^T @ E            # softmax denominator row [1, 512 i]
      s  = gamma / d                # [1, 512]
      bs = ones_col^T @ s           # PE broadcast across partitions
      out_c = o_c * bs + (kv + gamma*bh)   # DVE, then DMA out
"""

import os
import numpy as np

P = 128          # partitions
N = 4096         # tokens (H*W)
CA = 32          # attention channels
C = 256          # kv channels
IT = 512         # i-tile (query tokens per tile)
NI = N // IT     # 8
NJ = N // P      # 32
NCORES = 8

_cache = {}


def _split_multi_waits(nc, keep=1):
    """This walrus build encodes at most one sem wait per instruction
    (setupSyncWait: 'Too many sync wait commands').  Tile's sem assignment
    can attach several.  Move excess waits onto single-wait NoOps emitted
    just before the instruction on the same engine (engines execute their
    stream in order, so the waits still gate the instruction)."""
    import concourse.mybir as mybir
    import bass_rust

    for fn in nc.m.functions:
        for blk in fn.blocks:
            out = []
            for inst in blk.instructions:
                si = inst.sync_info
                if si is not None and len(si.on_wait) > keep:
                    waits = list(si.on_wait)
                    for k, w in enumerate(waits[:-keep]):
                        nop = mybir.InstNoOp(
                            name=f"{inst.name}_prewait{k}", ins=[], outs=[]
                        )
                        nop.engine = inst.engine
                        nop.sync_info = bass_rust.SyncInfo(on_wait=[w], on_update=[])
                        out.append(nop)
                    inst.sync_info = bass_rust.SyncInfo(
                        on_wait=waits[-keep:], on_update=list(si.on_update)
                    )
                out.append(inst)
            blk.instructions = out


def _build():
    import concourse.bass as bass
    import concourse.mybir as mybir
    from concourse.tile import TileContext
    from concourse.bass import ts

    f32 = mybir.dt.float32
    bf16 = mybir.dt.bfloat16
    AF = mybir.ActivationFunctionType

    nc = bass.Bass()
    q_d = nc.dram_tensor("q", [2, P, N], f32, kind="ExternalInput")
    kv_d = nc.dram_tensor("kv", [2, P, N], f32, kind="ExternalInput")
    wft_d = nc.dram_tensor("wft", [2, P, CA], f32, kind="ExternalInput")
    wgt_d = nc.dram_tensor("wgt", [2, P, CA], f32, kind="ExternalInput")
    wht_d = nc.dram_tensor("wht", [2, P, C], f32, kind="ExternalInput")
    bf_d = nc.dram_tensor("bfv", [CA, 1], f32, kind="ExternalInput")
    bg_d = nc.dram_tensor("bgv", [CA, 1], f32, kind="ExternalInput")
    bh_d = nc.dram_tensor("bhv", [P, 2], f32, kind="ExternalInput")
    gm_d = nc.dram_tensor("gam", [1, 1], f32, kind="ExternalInput")
    out_d = nc.dram_tensor("out", [2, P, N], f32, kind="ExternalOutput")

    with TileContext(nc) as tc:
        with (
            tc.tile_pool(name="const", bufs=1) as const,
            tc.tile_pool(name="big", bufs=1) as big,
        ):
            # ---- constants / small params ----
            wft = const.tile([P, 2, CA], f32)
            wgt = const.tile([P, 2, CA], f32)
            wht = const.tile([P, 2, C], f32)
            for k in range(2):
                nc.sync.dma_start(out=wft[:, k, :], in_=wft_d[k, :, :])
                nc.sync.dma_start(out=wgt[:, k, :], in_=wgt_d[k, :, :])
                nc.sync.dma_start(out=wht[:, k, :], in_=wht_d[k, :, :])
            bfs = const.tile([CA, 1], f32)
            bgs = const.tile([CA, 1], f32)
            nc.sync.dma_start(out=bfs, in_=bf_d[:, :])
            nc.sync.dma_start(out=bgs, in_=bg_d[:, :])
            bhs = const.tile([P, 2], f32)
            nc.sync.dma_start(out=bhs, in_=bh_d[:, :])
            gam = const.tile([P, 1], f32)
            nc.sync.dma_start(out=gam, in_=gm_d[:, :].to_broadcast([P, 1]))
            ones_col = const.tile([P, 1], bf16)
            nc.vector.memset(ones_col, 1.0)
            ones_row = const.tile([1, P], f32)
            nc.vector.memset(ones_row, 1.0)

            # gbh = gamma * bh  (per-partition, [128, 2])
            gbh = const.tile([P, 2], f32)
            nc.vector.tensor_scalar_mul(gbh, bhs, gam)

            # ---- big SBUF residents ----
            kv_sb = big.tile([P, 2, N], f32)
            kvgb = big.tile([P, 2, N], f32)  # kv + gamma*bh
            # fT4: fT replicated into 4 row strips (partitions 32s..32s+31)
            # so 4 ST matmuls (K=32) can run concurrently via row tiling.
            # g4: strip s, block k holds g j-tile (4k+s).
            fT4 = big.tile([P, N], bf16)
            g4 = big.tile([P, (NJ // 4) * P], bf16)
            g = big.tile([CA, N], bf16)
            h = big.tile([P, NJ, C], bf16)  # [token-in-jtile, jtile, channel]
            fT = fT4[0:CA, :]

            # bf16 copies of the weights for the projection matmuls
            wgt_bf = const.tile([P, 2, CA], bf16)
            wft_bf = const.tile([P, 2, CA], bf16)
            wht_bf = const.tile([P, 2, C], bf16)
            nc.vector.tensor_copy(out=wgt_bf, in_=wgt)
            nc.vector.tensor_copy(out=wft_bf, in_=wft)
            nc.vector.tensor_copy(out=wht_bf, in_=wht)

            # ---- chunked input DMA + bf16 casts + projections ----
            # Inputs arrive in [128, 512] pieces so casts and projection
            # matmuls start as soon as the first piece lands instead of
            # waiting for the full 4 MB transfer.
            with (
                tc.tile_pool(name="qpool", bufs=1) as qpool,
                tc.tile_pool(name="projps", bufs=2, space="PSUM") as pj,
            ):
                kv_bf = qpool.tile([P, 2, N], bf16)
                q_bf = qpool.tile([P, 2, N], bf16)
                q_sb = qpool.tile([P, 2, N], f32)

                for t in range(NI):
                    for k in range(2):
                        nc.sync.dma_start(
                            out=kv_sb[:, k, ts(t, IT)], in_=kv_d[k, :, ts(t, IT)]
                        )
                        # alternate cast between ACT and DVE
                        if k == 0:
                            nc.scalar.copy(
                                out=kv_bf[:, k, ts(t, IT)], in_=kv_sb[:, k, ts(t, IT)]
                            )
                        else:
                            nc.vector.tensor_copy(
                                out=kv_bf[:, k, ts(t, IT)], in_=kv_sb[:, k, ts(t, IT)]
                            )
                    # g projection for this i-slice
                    ps = pj.tile([CA, IT], f32, tag="fg", name=f"psg_{t}")
                    nc.tensor.matmul(
                        ps, lhsT=wgt_bf[:, 0, :], rhs=kv_bf[:, 0, ts(t, IT)],
                        start=True, stop=False,
                    )
                    nc.tensor.matmul(
                        ps, lhsT=wgt_bf[:, 1, :], rhs=kv_bf[:, 1, ts(t, IT)],
                        start=False, stop=True,
                    )
                    nc.scalar.activation(
                        out=g[:, ts(t, IT)], in_=ps, func=AF.Identity, bias=bgs
                    )
                    # h projection for the 4 j-tiles in this slice
                    for j in range(4 * t, 4 * t + 4):
                        ph = pj.tile([P, C], f32, tag="h", name=f"psh_{j}")
                        nc.tensor.matmul(
                            ph, lhsT=kv_bf[:, 0, ts(j, P)], rhs=wht_bf[:, 0, :],
                            start=True, stop=False,
                        )
                        nc.tensor.matmul(
                            ph, lhsT=kv_bf[:, 1, ts(j, P)], rhs=wht_bf[:, 1, :],
                            start=False, stop=True,
                        )
                        nc.vector.tensor_copy(out=h[:, j, :], in_=ph)
                    # kvgb pieces (fp32, exact)
                    for k in range(2):
                        nc.scalar.activation(
                            out=kvgb[:, k, ts(t, IT)],
                            in_=kv_sb[:, k, ts(t, IT)],
                            func=AF.Identity,
                            bias=gbh[:, k : k + 1],
                        )

                for t in range(NI):
                    for k in range(2):
                        nc.sync.dma_start(
                            out=q_sb[:, k, ts(t, IT)], in_=q_d[k, :, ts(t, IT)]
                        )
                        if k == 0:
                            nc.scalar.copy(
                                out=q_bf[:, k, ts(t, IT)], in_=q_sb[:, k, ts(t, IT)]
                            )
                        else:
                            nc.vector.tensor_copy(
                                out=q_bf[:, k, ts(t, IT)], in_=q_sb[:, k, ts(t, IT)]
                            )
                    ps = pj.tile([CA, IT], f32, tag="fg", name=f"psf_{t}")
                    nc.tensor.matmul(
                        ps, lhsT=wft_bf[:, 0, :], rhs=q_bf[:, 0, ts(t, IT)],
                        start=True, stop=False,
                    )
                    nc.tensor.matmul(
                        ps, lhsT=wft_bf[:, 1, :], rhs=q_bf[:, 1, ts(t, IT)],
                        start=False, stop=True,
                    )
                    nc.scalar.activation(
                        out=fT[:, ts(t, IT)], in_=ps, func=AF.Identity, bias=bfs
                    )

                # replicate fT into row strips 1..3 and scatter g into g4
                for s in range(1, 4):
                    nc.sync.dma_start(
                        out=fT4[CA * s : CA * (s + 1), :], in_=fT
                    )
                g_blk = g.rearrange("p (k four c) -> p k four c", four=4, c=P)
                g4_blk = g4.rearrange("p (k c) -> p k c", c=P)
                for s in range(4):
                    nc.sync.dma_start(
                        out=g4_blk[CA * s : CA * (s + 1), :, :],
                        in_=g_blk[:, :, s, :],
                    )

            # ---- main attention loop ----
            # j-tiles processed in packs of 4: the 4 ST matmuls (K=32) run
            # concurrently in the PE's four 32-row groups, writing 4 PSUM
            # banks of one [128, 4*512] tile; one ACTIVATE exps all 2048
            # columns.  PSUM banks: st4 4, o0/o1 bufs=1 = 2, d 1, bc 1 -> 8.
            with (
                tc.tile_pool(name="stps", bufs=1, space="PSUM") as stp,
                tc.tile_pool(name="ops", bufs=1, space="PSUM") as op,
                tc.tile_pool(name="dps", bufs=1, space="PSUM") as dp,
                tc.tile_pool(name="bcps", bufs=1, space="PSUM") as bcp,
                tc.tile_pool(name="epool", bufs=3) as ep,
                tc.tile_pool(name="spool", bufs=2) as sp,
                tc.tile_pool(name="outp", bufs=2) as outp,
            ):
                NP4 = NJ // 4  # packs per i-tile

                def st_exp_pack(t, k):
                    st4 = stp.tile([P, 4, IT], f32, tag="st4", name=f"st4_{t}_{k}")
                    for s in range(4):
                        nc.tensor.matmul(
                            st4[:, s, :],
                            lhsT=g4[CA * s : CA * (s + 1), ts(k, P)],
                            rhs=fT4[CA * s : CA * (s + 1), ts(t, IT)],
                            start=True, stop=True,
                            tile_position=(CA * s, 0),
                        )
                    E4 = ep.tile([P, 4, IT], bf16, tag="e4", name=f"e4_{t}_{k}")
                    nc.scalar.activation(out=E4, in_=st4, func=AF.Exp)
                    return E4

                def flush_pack(t, k, E4, o0, o1, dd):
                    first, last = k == 0, k == NP4 - 1
                    for s in range(4):
                        j = 4 * k + s
                        nc.tensor.matmul(
                            o0, lhsT=h[:, j, 0:P], rhs=E4[:, s, :],
                            start=first and s == 0, stop=last and s == 3,
                        )
                        nc.tensor.matmul(
                            o1, lhsT=h[:, j, P:C], rhs=E4[:, s, :],
                            start=first and s == 0, stop=last and s == 3,
                        )
                    # d-matmuls last, back-to-back: they share the stationary
                    # `ones` operand so only the first needs a weight load.
                    for s in range(4):
                        nc.tensor.matmul(
                            dd, lhsT=ones_col, rhs=E4[:, s, :],
                            start=first and s == 0, stop=last and s == 3,
                        )

                def epilogue(o0, o1, dd, t):
                    d_sb = sp.tile([1, IT], f32, tag="dsb")
                    nc.vector.tensor_copy(out=d_sb, in_=dd[0:1, :])
                    bc = bcp.tile([P, IT], f32, tag="bc")
                    nc.tensor.matmul(
                        bc, lhsT=ones_row, rhs=d_sb, start=True, stop=True
                    )
                    t1s = []
                    for k, ok in enumerate((o0, o1)):
                        t1 = outp.tile(
                            [P, IT], f32, tag=f"out{k}", name=f"t1_{t}_{k}"
                        )
                        # release the o psum bank promptly (gamma scale only)
                        nc.vector.tensor_scalar_mul(t1, ok, gam)
                        t1s.append(t1)
                    rbc = sp.tile([P, IT], f32, tag="rbc")
                    nc.vector.reciprocal(rbc, bc)
                    for k, t1 in enumerate(t1s):
                        nc.vector.scalar_tensor_tensor(
                            out=t1, in0=t1, scalar=1.0, in1=rbc,
                            op0=mybir.AluOpType.mult, op1=mybir.AluOpType.mult,
                        )
                        nc.vector.tensor_add(t1, t1, kvgb[:, k, ts(t, IT)])
                        nc.sync.dma_start(out=out_d[k, :, ts(t, IT)], in_=t1)

                packs = [(t, k) for t in range(NI) for k in range(NP4)]
                cur = {}
                pendingE = None
                pending_ep = None
                for t, k in packs:
                    if k == 0:
                        cur[t] = (
                            op.tile([P, IT], f32, tag="o0", name=f"o0_{t}"),
                            op.tile([P, IT], f32, tag="o1", name=f"o1_{t}"),
                            dp.tile([1, IT], f32, tag="dd", name=f"dd_{t}"),
                        )
                    E4 = st_exp_pack(t, k)
                    if pendingE is not None:
                        pt, pk, pE4 = pendingE
                        o0, o1, dd = cur[pt]
                        flush_pack(pt, pk, pE4, o0, o1, dd)
                        if pending_ep is not None and pk == 1:
                            epilogue(*pending_ep)
                            pending_ep = None
                        if pk == NP4 - 1:
                            pending_ep = (o0, o1, dd, pt)
                            del cur[pt]
                    pendingE = (t, k, E4)
                pt, pk, pE4 = pendingE
                o0, o1, dd = cur[pt]
                flush_pack(pt, pk, pE4, o0, o1, dd)
                if pending_ep is not None:
                    epilogue(*pending_ep)
                epilogue(o0, o1, dd, pt)

    _split_multi_waits(nc)
    return nc


def _get_nc():
    if "nc" not in _cache:
        _cache["nc"] = _build()
    return _cache["nc"]


def kernel(
    query_input, key_value_input, Wf, bf, Wg, bg, Wh, bh, gamma
):
    from concourse.bass_utils import run_bass_kernel_spmd

    B = query_input.shape[0]
    assert B == NCORES

    nc = _get_nc()

    f32 = np.float32
    wft = np.ascontiguousarray(Wf.T.reshape(2, P, CA), dtype=f32)
    wgt = np.ascontiguousarray(Wg.T.reshape(2, P, CA), dtype=f32)
    wht = np.ascontiguousarray(Wh.T.reshape(2, P, C), dtype=f32)
    bfv = np.ascontiguousarray(bf.reshape(CA, 1), dtype=f32)
    bgv = np.ascontiguousarray(bg.reshape(CA, 1), dtype=f32)
    bhv = np.ascontiguousarray(bh.reshape(2, P).T, dtype=f32)
    gm = np.ascontiguousarray(gamma.reshape(1, 1), dtype=f32)

    in_maps = []
    for b in range(B):
        in_maps.append(
            {
                "q": np.ascontiguousarray(
                    query_input[b].reshape(2, P, N), dtype=f32
                ),
                "kv": np.ascontiguousarray(
                    key_value_input[b].reshape(2, P, N), dtype=f32
                ),
                "wft": wft,
                "wgt": wgt,
                "wht": wht,
                "bfv": bfv,
                "bgv": bgv,
                "bhv": bhv,
                "gam": gm,
            }
        )

    res = run_bass_kernel_spmd(nc, in_maps, core_ids=list(range(NCORES)))
    _cache["last_result"] = res
    out = np.empty((B, C, 64, 64), dtype=f32)
    for b in range(B):
        out[b] = res.results[b]["out"].reshape(C, 64, 64)
    return out


if __name__ == "__main__":
    rng = np.random.default_rng(0)
    inputs = {
        "query_input": rng.standard_normal((8, 256, 64, 64), dtype=np.float32),
        "key_value_input": rng.standard_normal((8, 256, 64, 64), dtype=np.float32),
        "Wf": rng.standard_normal((CA, C), dtype=np.float32) * 0.06,
        "bf": rng.standard_normal((CA,), dtype=np.float32) * 0.06,
        "Wg": rng.standard_normal((CA, C), dtype=np.float32) * 0.06,
        "bg": rng.standard_normal((CA,), dtype=np.float32) * 0.06,
        "Wh": rng.standard_normal((C, C), dtype=np.float32) * 0.06,
        "bh": rng.standard_normal((C,), dtype=np.float32) * 0.06,
        "gamma": np.zeros((1,), dtype=np.float32),
    }
    out = kernel(**inputs)
    print(out.shape, out.dtype)


# revision 18
# speedup vs baseline: 1.8980x; 1.1471x over previous
"""Trainium2 Bass kernel for SAGAN-style 2D self-attention (nn_Attention2d).

Reference computation (per batch element b):
    q  = query_input[b].reshape(Cq, N)          # N = H*W = 4096, Cq = 256
    kv = key_value_input[b].reshape(C, N)       # C = 256
    fT = Wf @ q + bf        # [32, N]   (f transposed)
    g  = Wg @ kv + bg       # [32, N]
    h  = (Wh @ kv + bh).T   # [N, C]
    beta = softmax(fT.T @ g, axis=-1)           # [N, N]
    o  = beta @ h                               # [N, C]
    out[b] = gamma * o.T + kv                   # [C, N] -> [C, H, W]

Sharding: data-parallel over batch, one batch element per NeuronCore (B=8,
8 cores, no collectives).

Per-core algorithm (all layouts transposed so softmax reductions ride the
matmul path; no on-chip transposes needed):
  - projections: fT [32,N], g [32,N] (token-major), h [N,C] (token, channel)
  - loop over i-tiles (512 query tokens):
      for each j-tile (128 key tokens):
        ST  = g_j^T @ fT_i          # [128 j, 512 i] logits, PSUM
        E   = exp(ST)               # ACT, bf16 -> SBUF (no max subtraction:
                                    #  logits are bounded ~ +-13 for this op)
        o0 += h_j[:, 0:128]^T @ E   # accumulate over j in PSUM  [128 c, 512 i]
        o1 += h_j[:,128:256]^T @ E
        d  += ones^T @ E            # softmax denominator row [1, 512 i]
      s  = gamma / d                # [1, 512]
      bs = ones_col^T @ s           # PE broadcast across partitions
      out_c = o_c * bs + (kv + gamma*bh)   # DVE, then DMA out
"""

import os
import numpy as np

P = 128          # partitions
N = 4096         # tokens (H*W)
CA = 32          # attention channels
C = 256          # kv channels
IT = 512         # i-tile (query tokens per tile)
NI = N // IT     # 8
NJ = N // P      # 32
NCORES = 8

_cache = {}


def _split_multi_waits(nc, keep=1):
    """This walrus build encodes at most one sem wait per instruction
    (setupSyncWait: 'Too many sync wait commands').  Tile's sem assignment
    can attach several.  Move excess waits onto single-wait NoOps emitted
    just before the instruction on the same engine (engines execute their
    stream in order, so the waits still gate the instruction)."""
    import concourse.mybir as mybir
    import bass_rust

    for fn in nc.m.functions:
        for blk in fn.blocks:
            out = []
            for inst in blk.instructions:
                si = inst.sync_info
                if si is not None and len(si.on_wait) > keep:
                    waits = list(si.on_wait)
                    for k, w in enumerate(waits[:-keep]):
                        nop = mybir.InstNoOp(
                            name=f"{inst.name}_prewait{k}", ins=[], outs=[]
                        )
                        nop.engine = inst.engine
                        nop.sync_info = bass_rust.SyncInfo(on_wait=[w], on_update=[])
                        out.append(nop)
                    inst.sync_info = bass_rust.SyncInfo(
                        on_wait=waits[-keep:], on_update=list(si.on_update)
                    )
                out.append(inst)
            blk.instructions = out


def _build():
    import concourse.bass as bass
    import concourse.mybir as mybir
    from concourse.tile import TileContext
    from concourse.bass import ts

    f32 = mybir.dt.float32
    bf16 = mybir.dt.bfloat16
    AF = mybir.ActivationFunctionType

    nc = bass.Bass()
    q_d = nc.dram_tensor("q", [2, P, N], f32, kind="ExternalInput")
    kv_d = nc.dram_tensor("kv", [2, P, N], f32, kind="ExternalInput")
    # all small params packed into one tensor (single DMA):
    # cols [0:64]  wft (2 chunks of 32), [64:128] wgt, [128:640] wht,
    # cols [640:642] bh (2 chunks), [642] gamma (pre-broadcast by host),
    # col  [643] bf (rows 0:32), col [644] bg (rows 0:32)
    par_d = nc.dram_tensor("par", [P, 645], f32, kind="ExternalInput")
    out_d = nc.dram_tensor("out", [2, P, N], f32, kind="ExternalOutput")

    with TileContext(nc) as tc:
        with (
            tc.tile_pool(name="const", bufs=1) as const,
            tc.tile_pool(name="big", bufs=1) as big,
        ):
            # ---- constants / small params ----
            par = const.tile([P, 645], f32)
            nc.sync.dma_start(out=par, in_=par_d[:, :])
            wft = par[:, 0:64].rearrange("p (k a) -> p k a", k=2)
            wgt = par[:, 64:128].rearrange("p (k a) -> p k a", k=2)
            wht = par[:, 128:640].rearrange("p (k a) -> p k a", k=2)
            bhs = par[:, 640:642]
            gam = par[:, 642:643]
            bfs = par[0:CA, 643:644]
            bgs = par[0:CA, 644:645]
            ones_col = const.tile([P, 1], bf16)
            nc.vector.memset(ones_col, 1.0)
            # mask4: rows 0/32/64/96 are ones -> lhsT that sums the four
            # d-partial rows and broadcasts the result to all 128 partitions
            mask4 = const.tile([P, P], f32)
            nc.vector.memset(mask4, 0.0)
            for s in range(4):
                nc.vector.memset(mask4[32 * s : 32 * s + 1, :], 1.0)

            # gbh = gamma * bh  (per-partition, [128, 2])
            gbh = const.tile([P, 2], f32)
            nc.vector.tensor_scalar_mul(gbh, bhs, gam)

            # ---- big SBUF residents ----
            kv_sb = big.tile([P, 2, N], f32)
            kvgb = big.tile([P, 2, N], f32)  # kv + gamma*bh
            # fT4: fT replicated into 4 row strips (partitions 32s..32s+31)
            # so 4 ST matmuls (K=32) can run concurrently via row tiling.
            # g4: strip s, block k holds g j-tile (4k+s).
            fT4 = big.tile([P, N], bf16)
            g4 = big.tile([P, (NJ // 4) * P], bf16)
            g = big.tile([CA, N], bf16)
            h = big.tile([P, NJ, C], bf16)  # [token-in-jtile, jtile, channel]
            fT = fT4[0:CA, :]

            # bf16 copies of the weights for the projection matmuls
            wgt_bf = const.tile([P, 2, CA], bf16)
            wft_bf = const.tile([P, 2, CA], bf16)
            wht_bf = const.tile([P, 2, C], bf16)
            nc.vector.tensor_copy(out=wgt_bf, in_=wgt)
            nc.vector.tensor_copy(out=wft_bf, in_=wft)
            nc.vector.tensor_copy(out=wht_bf, in_=wht)

            # ---- chunked input DMA + bf16 casts + projections ----
            # Inputs arrive in [128, 512] pieces so casts and projection
            # matmuls start as soon as the first piece lands instead of
            # waiting for the full 4 MB transfer.
            with (
                tc.tile_pool(name="qpool", bufs=1) as qpool,
                tc.tile_pool(name="projps", bufs=2, space="PSUM") as pj,
            ):
                kv_bf = qpool.tile([P, 2, N], bf16)
                q_bf = qpool.tile([P, 2, N], bf16)
                q_sb = qpool.tile([P, 2, N], f32)

                QW = 2 * IT  # 1 MB quarters
                for tq in range(N // QW):
                    for k in range(2):
                        nc.sync.dma_start(
                            out=kv_sb[:, k, ts(tq, QW)], in_=kv_d[k, :, ts(tq, QW)]
                        )
                for t in range(NI):
                    for k in range(2):
                        # alternate cast between ACT and DVE
                        if k == 0:
                            nc.scalar.copy(
                                out=kv_bf[:, k, ts(t, IT)], in_=kv_sb[:, k, ts(t, IT)]
                            )
                        else:
                            nc.vector.tensor_copy(
                                out=kv_bf[:, k, ts(t, IT)], in_=kv_sb[:, k, ts(t, IT)]
                            )
                    # g projection for this i-slice
                    ps = pj.tile([CA, IT], f32, tag="fg", name=f"psg_{t}")
                    nc.tensor.matmul(
                        ps, lhsT=wgt_bf[:, 0, :], rhs=kv_bf[:, 0, ts(t, IT)],
                        start=True, stop=False,
                    )
                    nc.tensor.matmul(
                        ps, lhsT=wgt_bf[:, 1, :], rhs=kv_bf[:, 1, ts(t, IT)],
                        start=False, stop=True,
                    )
                    nc.scalar.activation(
                        out=g[:, ts(t, IT)], in_=ps, func=AF.Identity, bias=bgs
                    )
                    # h projection for the 4 j-tiles in this slice
                    for j in range(4 * t, 4 * t + 4):
                        ph = pj.tile([P, C], f32, tag="h", name=f"psh_{j}")
                        nc.tensor.matmul(
                            ph, lhsT=kv_bf[:, 0, ts(j, P)], rhs=wht_bf[:, 0, :],
                            start=True, stop=False,
                        )
                        nc.tensor.matmul(
                            ph, lhsT=kv_bf[:, 1, ts(j, P)], rhs=wht_bf[:, 1, :],
                            start=False, stop=True,
                        )
                        nc.vector.tensor_copy(out=h[:, j, :], in_=ph)
                    # kvgb pieces (fp32, exact)
                    for k in range(2):
                        nc.scalar.activation(
                            out=kvgb[:, k, ts(t, IT)],
                            in_=kv_sb[:, k, ts(t, IT)],
                            func=AF.Identity,
                            bias=gbh[:, k : k + 1],
                        )

                for tq in range(N // QW):
                    for k in range(2):
                        nc.sync.dma_start(
                            out=q_sb[:, k, ts(tq, QW)], in_=q_d[k, :, ts(tq, QW)]
                        )
                for t in range(NI):
                    for k in range(2):
                        if k == 0:
                            nc.scalar.copy(
                                out=q_bf[:, k, ts(t, IT)], in_=q_sb[:, k, ts(t, IT)]
                            )
                        else:
                            nc.vector.tensor_copy(
                                out=q_bf[:, k, ts(t, IT)], in_=q_sb[:, k, ts(t, IT)]
                            )
                    ps = pj.tile([CA, IT], f32, tag="fg", name=f"psf_{t}")
                    nc.tensor.matmul(
                        ps, lhsT=wft_bf[:, 0, :], rhs=q_bf[:, 0, ts(t, IT)],
                        start=True, stop=False,
                    )
                    nc.tensor.matmul(
                        ps, lhsT=wft_bf[:, 1, :], rhs=q_bf[:, 1, ts(t, IT)],
                        start=False, stop=True,
                    )
                    nc.scalar.activation(
                        out=fT[:, ts(t, IT)], in_=ps, func=AF.Identity, bias=bfs
                    )

                # replicate fT into row strips 1..3 and scatter g into g4
                for s in range(1, 4):
                    nc.gpsimd.dma_start(
                        out=fT4[CA * s : CA * (s + 1), :], in_=fT
                    )
                g_blk = g.rearrange("p (k four c) -> p k four c", four=4, c=P)
                g4_blk = g4.rearrange("p (k c) -> p k c", c=P)
                for s in range(4):
                    nc.gpsimd.dma_start(
                        out=g4_blk[CA * s : CA * (s + 1), :, :],
                        in_=g_blk[:, :, s, :],
                    )

            # ---- main attention loop ----
            # j-tiles processed in packs of 4: the 4 ST matmuls (K=32) run
            # concurrently in the PE's four 32-row groups, writing 4 PSUM
            # banks of one [128, 4*512] tile; one ACTIVATE exps all 2048
            # columns.  PSUM banks: st4 4, o0/o1 bufs=1 = 2, d 1, bc 1 -> 8.
            with (
                tc.tile_pool(name="stps", bufs=1, space="PSUM") as stp,
                tc.tile_pool(name="ops", bufs=1, space="PSUM") as op,
                tc.tile_pool(name="dps", bufs=1, space="PSUM") as dp,
                tc.tile_pool(name="bcps", bufs=1, space="PSUM") as bcp,
                tc.tile_pool(name="epool", bufs=3) as ep,
                tc.tile_pool(name="spool", bufs=2) as sp,
                tc.tile_pool(name="outp", bufs=2) as outp,
            ):
                NP4 = NJ // 4  # packs per i-tile

                def st_exp_pack(t, k):
                    st4 = stp.tile([P, 4, IT], f32, tag="st4", name=f"st4_{t}_{k}")
                    for s in range(4):
                        nc.tensor.matmul(
                            st4[:, s, :],
                            lhsT=g4[CA * s : CA * (s + 1), ts(k, P)],
                            rhs=fT4[CA * s : CA * (s + 1), ts(t, IT)],
                            start=True, stop=True,
                            tile_position=(CA * s, 0),
                        )
                    E4 = ep.tile([P, 4, IT], bf16, tag="e4", name=f"e4_{t}_{k}")
                    nc.scalar.activation(out=E4, in_=st4, func=AF.Exp)
                    return E4

                def flush_pack(t, k, E4, o0, o1, dd):
                    first, last = k == 0, k == NP4 - 1
                    for s in range(4):
                        j = 4 * k + s
                        nc.tensor.matmul(
                            o0, lhsT=h[:, j, 0:P], rhs=E4[:, s, :],
                            start=first and s == 0, stop=last and s == 3,
                        )
                        nc.tensor.matmul(
                            o1, lhsT=h[:, j, P:C], rhs=E4[:, s, :],
                            start=first and s == 0, stop=last and s == 3,
                        )
                    # d-matmuls col-packed: 4 concurrent M=1 matmuls in the
                    # four 32-column PE groups, each writing one row (partition
                    # 32s) of the shared dd bank.
                    for s in range(4):
                        nc.tensor.matmul(
                            dd[32 * s : 32 * s + 1, :], lhsT=ones_col,
                            rhs=E4[:, s, :],
                            start=first, stop=last,
                            tile_position=(0, 32 * s),
                        )

                def epilogue(o0, o1, dd, t):
                    # copy the d bank to SBUF; the bc matmul (lhsT=mask4)
                    # sums the four partial rows (partitions 0/32/64/96) while
                    # broadcasting the result across all 128 partitions.
                    d_sb = sp.tile([P, IT], f32, tag="dsb")
                    nc.vector.tensor_copy(out=d_sb, in_=dd)
                    bc = bcp.tile([P, IT], f32, tag="bc")
                    nc.tensor.matmul(
                        bc, lhsT=mask4, rhs=d_sb, start=True, stop=True
                    )
                    t1s = []
                    for k, ok in enumerate((o0, o1)):
                        t1 = outp.tile(
                            [P, IT], f32, tag=f"out{k}", name=f"t1_{t}_{k}"
                        )
                        # release the o psum bank promptly (gamma scale only)
                        nc.vector.tensor_scalar_mul(t1, ok, gam)
                        t1s.append(t1)
                    rbc = sp.tile([P, IT], f32, tag="rbc")
                    nc.vector.reciprocal(rbc, bc)
                    for k, t1 in enumerate(t1s):
                        nc.vector.scalar_tensor_tensor(
                            out=t1, in0=t1, scalar=1.0, in1=rbc,
                            op0=mybir.AluOpType.mult, op1=mybir.AluOpType.mult,
                        )
                        nc.vector.tensor_add(t1, t1, kvgb[:, k, ts(t, IT)])
                        nc.sync.dma_start(out=out_d[k, :, ts(t, IT)], in_=t1)

                packs = [(t, k) for t in range(NI) for k in range(NP4)]
                cur = {}
                pendingE = None
                pending_ep = None
                for t, k in packs:
                    if k == 0:
                        cur[t] = (
                            op.tile([P, IT], f32, tag="o0", name=f"o0_{t}"),
                            op.tile([P, IT], f32, tag="o1", name=f"o1_{t}"),
                            dp.tile([P, IT], f32, tag="dd", name=f"dd_{t}"),
                        )
                    E4 = st_exp_pack(t, k)
                    if pendingE is not None:
                        pt, pk, pE4 = pendingE
                        o0, o1, dd = cur[pt]
                        flush_pack(pt, pk, pE4, o0, o1, dd)
                        if pending_ep is not None and pk == 1:
                            epilogue(*pending_ep)
                            pending_ep = None
                        if pk == NP4 - 1:
                            pending_ep = (o0, o1, dd, pt)
                            del cur[pt]
                    pendingE = (t, k, E4)
                pt, pk, pE4 = pendingE
                o0, o1, dd = cur[pt]
                flush_pack(pt, pk, pE4, o0, o1, dd)
                if pending_ep is not None:
                    epilogue(*pending_ep)
                epilogue(o0, o1, dd, pt)

    _split_multi_waits(nc)
    return nc


def _get_nc():
    if "nc" not in _cache:
        _cache["nc"] = _build()
    return _cache["nc"]


def kernel(
    query_input, key_value_input, Wf, bf, Wg, bg, Wh, bh, gamma
):
    from concourse.bass_utils import run_bass_kernel_spmd

    B = query_input.shape[0]
    assert B == NCORES

    nc = _get_nc()

    f32 = np.float32
    par = np.zeros((P, 645), dtype=f32)
    par[:, 0:64] = Wf.T.reshape(2, P, CA).transpose(1, 0, 2).reshape(P, 64)
    par[:, 64:128] = Wg.T.reshape(2, P, CA).transpose(1, 0, 2).reshape(P, 64)
    par[:, 128:640] = Wh.T.reshape(2, P, C).transpose(1, 0, 2).reshape(P, 512)
    par[:, 640:642] = bh.reshape(2, P).T
    par[:, 642] = np.float32(gamma.reshape(-1)[0])
    par[0:CA, 643] = bf.reshape(CA)
    par[0:CA, 644] = bg.reshape(CA)
    par = np.ascontiguousarray(par)

    in_maps = []
    for b in range(B):
        in_maps.append(
            {
                "q": np.ascontiguousarray(
                    query_input[b].reshape(2, P, N), dtype=f32
                ),
                "kv": np.ascontiguousarray(
                    key_value_input[b].reshape(2, P, N), dtype=f32
                ),
                "par": par,
            }
        )

    res = run_bass_kernel_spmd(nc, in_maps, core_ids=list(range(NCORES)))
    _cache["last_result"] = res
    out = np.empty((B, C, 64, 64), dtype=f32)
    for b in range(B):
        out[b] = res.results[b]["out"].reshape(C, 64, 64)
    return out


if __name__ == "__main__":
    rng = np.random.default_rng(0)
    inputs = {
        "query_input": rng.standard_normal((8, 256, 64, 64), dtype=np.float32),
        "key_value_input": rng.standard_normal((8, 256, 64, 64), dtype=np.float32),
        "Wf": rng.standard_normal((CA, C), dtype=np.float32) * 0.06,
        "bf": rng.standard_normal((CA,), dtype=np.float32) * 0.06,
        "Wg": rng.standard_normal((CA, C), dtype=np.float32) * 0.06,
        "bg": rng.standard_normal((CA,), dtype=np.float32) * 0.06,
        "Wh": rng.standard_normal((C, C), dtype=np.float32) * 0.06,
        "bh": rng.standard_normal((C,), dtype=np.float32) * 0.06,
        "gamma": np.zeros((1,), dtype=np.float32),
    }
    out = kernel(**inputs)
    print(out.shape, out.dtype)


# revision 19
# speedup vs baseline: 1.9310x; 1.0174x over previous
"""Trainium2 Bass kernel for SAGAN-style 2D self-attention (nn_Attention2d).

Reference computation (per batch element b):
    q  = query_input[b].reshape(Cq, N)          # N = H*W = 4096, Cq = 256
    kv = key_value_input[b].reshape(C, N)       # C = 256
    fT = Wf @ q + bf        # [32, N]   (f transposed)
    g  = Wg @ kv + bg       # [32, N]
    h  = (Wh @ kv + bh).T   # [N, C]
    beta = softmax(fT.T @ g, axis=-1)           # [N, N]
    o  = beta @ h                               # [N, C]
    out[b] = gamma * o.T + kv                   # [C, N] -> [C, H, W]

Sharding: data-parallel over batch, one batch element per NeuronCore (B=8,
8 cores, no collectives).

Per-core algorithm (all layouts transposed so softmax reductions ride the
matmul path; no on-chip transposes needed):
  - projections: fT [32,N], g [32,N] (token-major), h [N,C] (token, channel)
  - loop over i-tiles (512 query tokens):
      for each j-tile (128 key tokens):
        ST  = g_j^T @ fT_i          # [128 j, 512 i] logits, PSUM
        E   = exp(ST)               # ACT, bf16 -> SBUF (no max subtraction:
                                    #  logits are bounded ~ +-13 for this op)
        o0 += h_j[:, 0:128]^T @ E   # accumulate over j in PSUM  [128 c, 512 i]
        o1 += h_j[:,128:256]^T @ E
        d  += ones^T @ E            # softmax denominator row [1, 512 i]
      s  = gamma / d                # [1, 512]
      bs = ones_col^T @ s           # PE broadcast across partitions
      out_c = o_c * bs + (kv + gamma*bh)   # DVE, then DMA out
"""

import os
import numpy as np

P = 128          # partitions
N = 4096         # tokens (H*W)
CA = 32          # attention channels
C = 256          # kv channels
IT = 512         # i-tile (query tokens per tile)
NI = N // IT     # 8
NJ = N // P      # 32
NCORES = 8

_cache = {}


def _split_multi_waits(nc, keep=1):
    """This walrus build encodes at most one sem wait per instruction
    (setupSyncWait: 'Too many sync wait commands').  Tile's sem assignment
    can attach several.  Move excess waits onto single-wait NoOps emitted
    just before the instruction on the same engine (engines execute their
    stream in order, so the waits still gate the instruction)."""
    import concourse.mybir as mybir
    import bass_rust

    for fn in nc.m.functions:
        for blk in fn.blocks:
            out = []
            for inst in blk.instructions:
                si = inst.sync_info
                if si is not None and len(si.on_wait) > keep:
                    waits = list(si.on_wait)
                    for k, w in enumerate(waits[:-keep]):
                        nop = mybir.InstNoOp(
                            name=f"{inst.name}_prewait{k}", ins=[], outs=[]
                        )
                        nop.engine = inst.engine
                        nop.sync_info = bass_rust.SyncInfo(on_wait=[w], on_update=[])
                        out.append(nop)
                    inst.sync_info = bass_rust.SyncInfo(
                        on_wait=waits[-keep:], on_update=list(si.on_update)
                    )
                out.append(inst)
            blk.instructions = out


def _build():
    import concourse.bass as bass
    import concourse.mybir as mybir
    from concourse.tile import TileContext
    from concourse.bass import ts

    f32 = mybir.dt.float32
    bf16 = mybir.dt.bfloat16
    AF = mybir.ActivationFunctionType

    nc = bass.Bass()
    q_d = nc.dram_tensor("q", [2, P, N], f32, kind="ExternalInput")
    kv_d = nc.dram_tensor("kv", [2, P, N], f32, kind="ExternalInput")
    # all small params packed into one tensor (single DMA):
    # cols [0:64]  wft (2 chunks of 32), [64:128] wgt, [128:640] wht,
    # cols [640:642] bh (2 chunks), [642] gamma (pre-broadcast by host),
    # col  [643] bf (rows 0:32), col [644] bg (rows 0:32)
    par_d = nc.dram_tensor("par", [P, 645], f32, kind="ExternalInput")
    out_d = nc.dram_tensor("out", [2, P, N], f32, kind="ExternalOutput")

    with TileContext(nc) as tc:
        with (
            tc.tile_pool(name="const", bufs=1) as const,
            tc.tile_pool(name="big", bufs=1) as big,
        ):
            # ---- constants / small params ----
            par = const.tile([P, 645], f32)
            nc.sync.dma_start(out=par, in_=par_d[:, :])
            wft = par[:, 0:64].rearrange("p (k a) -> p k a", k=2)
            wgt = par[:, 64:128].rearrange("p (k a) -> p k a", k=2)
            wht = par[:, 128:640].rearrange("p (k a) -> p k a", k=2)
            bhs = par[:, 640:642]
            gam = par[:, 642:643]
            bfs = par[0:CA, 643:644]
            bgs = par[0:CA, 644:645]
            ones_col = const.tile([P, 1], bf16)
            nc.vector.memset(ones_col, 1.0)
            # mask4: rows 0/32/64/96 are ones -> lhsT that sums the four
            # d-partial rows and broadcasts the result to all 128 partitions
            mask4 = const.tile([P, P], f32)
            nc.vector.memset(mask4, 0.0)
            for s in range(4):
                nc.vector.memset(mask4[32 * s : 32 * s + 1, :], 1.0)

            # gbh = gamma * bh  (per-partition, [128, 2])
            gbh = const.tile([P, 2], f32)
            nc.vector.tensor_scalar_mul(gbh, bhs, gam)

            # ---- big SBUF residents ----
            kv_sb = big.tile([P, 2, N], f32)
            kvgb = big.tile([P, 2, N], f32)  # kv + gamma*bh
            # fT4: fT replicated into 4 row strips (partitions 32s..32s+31)
            # so 4 ST matmuls (K=32) can run concurrently via row tiling.
            # g4: strip s, block k holds g j-tile (4k+s).
            fT4 = big.tile([P, N], bf16)
            g4 = big.tile([P, (NJ // 4) * P], bf16)
            g = big.tile([CA, N], bf16)
            h = big.tile([P, NJ, C], bf16)  # [token-in-jtile, jtile, channel]
            fT = fT4[0:CA, :]

            # bf16 copies of the weights for the projection matmuls
            wgt_bf = const.tile([P, 2, CA], bf16)
            wft_bf = const.tile([P, 2, CA], bf16)
            wht_bf = const.tile([P, 2, C], bf16)
            nc.vector.tensor_copy(out=wgt_bf, in_=wgt)
            nc.vector.tensor_copy(out=wft_bf, in_=wft)
            nc.vector.tensor_copy(out=wht_bf, in_=wht)

            # ---- chunked input DMA + bf16 casts + projections ----
            # Inputs arrive in [128, 512] pieces so casts and projection
            # matmuls start as soon as the first piece lands instead of
            # waiting for the full 4 MB transfer.
            with (
                tc.tile_pool(name="qpool", bufs=1) as qpool,
                tc.tile_pool(name="projps", bufs=2, space="PSUM") as pj,
            ):
                kv_bf = qpool.tile([P, 2, N], bf16)
                q_bf = qpool.tile([P, 2, N], bf16)
                q_sb = qpool.tile([P, 2, N], f32)

                QW = 2 * IT  # 1 MB quarters
                # interleave kv and q quarters so the f projection isn't
                # gated behind the full kv transfer
                for tq in range(N // QW):
                    for k in range(2):
                        nc.sync.dma_start(
                            out=kv_sb[:, k, ts(tq, QW)], in_=kv_d[k, :, ts(tq, QW)]
                        )
                    for k in range(2):
                        nc.sync.dma_start(
                            out=q_sb[:, k, ts(tq, QW)], in_=q_d[k, :, ts(tq, QW)]
                        )

                for t in range(NI):
                    # casts: fp32 -> bf16 (ACT for chunk 0, DVE for chunk 1)
                    nc.scalar.copy(
                        out=kv_bf[:, 0, ts(t, IT)], in_=kv_sb[:, 0, ts(t, IT)]
                    )
                    nc.vector.tensor_copy(
                        out=kv_bf[:, 1, ts(t, IT)], in_=kv_sb[:, 1, ts(t, IT)]
                    )
                    nc.scalar.copy(
                        out=q_bf[:, 0, ts(t, IT)], in_=q_sb[:, 0, ts(t, IT)]
                    )
                    nc.vector.tensor_copy(
                        out=q_bf[:, 1, ts(t, IT)], in_=q_sb[:, 1, ts(t, IT)]
                    )
                    # g projection for this i-slice
                    ps = pj.tile([CA, IT], f32, tag="fg", name=f"psg_{t}")
                    nc.tensor.matmul(
                        ps, lhsT=wgt_bf[:, 0, :], rhs=kv_bf[:, 0, ts(t, IT)],
                        start=True, stop=False,
                    )
                    nc.tensor.matmul(
                        ps, lhsT=wgt_bf[:, 1, :], rhs=kv_bf[:, 1, ts(t, IT)],
                        start=False, stop=True,
                    )
                    nc.scalar.activation(
                        out=g[:, ts(t, IT)], in_=ps, func=AF.Identity, bias=bgs
                    )
                    # f projection for this i-slice
                    psf = pj.tile([CA, IT], f32, tag="fg", name=f"psf_{t}")
                    nc.tensor.matmul(
                        psf, lhsT=wft_bf[:, 0, :], rhs=q_bf[:, 0, ts(t, IT)],
                        start=True, stop=False,
                    )
                    nc.tensor.matmul(
                        psf, lhsT=wft_bf[:, 1, :], rhs=q_bf[:, 1, ts(t, IT)],
                        start=False, stop=True,
                    )
                    nc.scalar.activation(
                        out=fT[:, ts(t, IT)], in_=psf, func=AF.Identity, bias=bfs
                    )
                    # h projection for the 4 j-tiles in this slice
                    for j in range(4 * t, 4 * t + 4):
                        ph = pj.tile([P, C], f32, tag="h", name=f"psh_{j}")
                        nc.tensor.matmul(
                            ph, lhsT=kv_bf[:, 0, ts(j, P)], rhs=wht_bf[:, 0, :],
                            start=True, stop=False,
                        )
                        nc.tensor.matmul(
                            ph, lhsT=kv_bf[:, 1, ts(j, P)], rhs=wht_bf[:, 1, :],
                            start=False, stop=True,
                        )
                        nc.vector.tensor_copy(out=h[:, j, :], in_=ph)
                    # kvgb pieces (fp32, exact) on DVE
                    for k in range(2):
                        nc.vector.tensor_scalar_add(
                            kvgb[:, k, ts(t, IT)],
                            kv_sb[:, k, ts(t, IT)],
                            gbh[:, k : k + 1],
                        )

                # replicate fT into row strips 1..3 and scatter g into g4
                for s in range(1, 4):
                    nc.gpsimd.dma_start(
                        out=fT4[CA * s : CA * (s + 1), :], in_=fT
                    )
                g_blk = g.rearrange("p (k four c) -> p k four c", four=4, c=P)
                g4_blk = g4.rearrange("p (k c) -> p k c", c=P)
                for s in range(4):
                    nc.gpsimd.dma_start(
                        out=g4_blk[CA * s : CA * (s + 1), :, :],
                        in_=g_blk[:, :, s, :],
                    )

            # ---- main attention loop ----
            # j-tiles processed in packs of 4: the 4 ST matmuls (K=32) run
            # concurrently in the PE's four 32-row groups, writing 4 PSUM
            # banks of one [128, 4*512] tile; one ACTIVATE exps all 2048
            # columns.  PSUM banks: st4 4, o0/o1 bufs=1 = 2, d 1, bc 1 -> 8.
            with (
                tc.tile_pool(name="stps", bufs=1, space="PSUM") as stp,
                tc.tile_pool(name="ops", bufs=1, space="PSUM") as op,
                tc.tile_pool(name="dps", bufs=1, space="PSUM") as dp,
                tc.tile_pool(name="bcps", bufs=1, space="PSUM") as bcp,
                tc.tile_pool(name="epool", bufs=3) as ep,
                tc.tile_pool(name="spool", bufs=2) as sp,
                tc.tile_pool(name="outp", bufs=2) as outp,
            ):
                NP4 = NJ // 4  # packs per i-tile

                def st_exp_pack(t, k):
                    st4 = stp.tile([P, 4, IT], f32, tag="st4", name=f"st4_{t}_{k}")
                    for s in range(4):
                        nc.tensor.matmul(
                            st4[:, s, :],
                            lhsT=g4[CA * s : CA * (s + 1), ts(k, P)],
                            rhs=fT4[CA * s : CA * (s + 1), ts(t, IT)],
                            start=True, stop=True,
                            tile_position=(CA * s, 0),
                        )
                    E4 = ep.tile([P, 4, IT], bf16, tag="e4", name=f"e4_{t}_{k}")
                    nc.scalar.activation(out=E4, in_=st4, func=AF.Exp)
                    return E4

                def flush_pack(t, k, E4, o0, o1, dd):
                    first, last = k == 0, k == NP4 - 1
                    for s in range(4):
                        j = 4 * k + s
                        nc.tensor.matmul(
                            o0, lhsT=h[:, j, 0:P], rhs=E4[:, s, :],
                            start=first and s == 0, stop=last and s == 3,
                        )
                        nc.tensor.matmul(
                            o1, lhsT=h[:, j, P:C], rhs=E4[:, s, :],
                            start=first and s == 0, stop=last and s == 3,
                        )
                    # d-matmuls col-packed: 4 concurrent M=1 matmuls in the
                    # four 32-column PE groups, each writing one row (partition
                    # 32s) of the shared dd bank.
                    for s in range(4):
                        nc.tensor.matmul(
                            dd[32 * s : 32 * s + 1, :], lhsT=ones_col,
                            rhs=E4[:, s, :],
                            start=first, stop=last,
                            tile_position=(0, 32 * s),
                        )

                def epilogue(o0, o1, dd, t):
                    # copy the d bank to SBUF; the bc matmul (lhsT=mask4)
                    # sums the four partial rows (partitions 0/32/64/96) while
                    # broadcasting the result across all 128 partitions.
                    d_sb = sp.tile([P, IT], f32, tag="dsb")
                    nc.vector.tensor_copy(out=d_sb, in_=dd)
                    bc = bcp.tile([P, IT], f32, tag="bc")
                    nc.tensor.matmul(
                        bc, lhsT=mask4, rhs=d_sb, start=True, stop=True
                    )
                    t1s = []
                    for k, ok in enumerate((o0, o1)):
                        t1 = outp.tile(
                            [P, IT], f32, tag=f"out{k}", name=f"t1_{t}_{k}"
                        )
                        # release the o psum bank promptly (gamma scale only)
                        nc.vector.tensor_scalar_mul(t1, ok, gam)
                        t1s.append(t1)
                    rbc = sp.tile([P, IT], f32, tag="rbc")
                    nc.vector.reciprocal(rbc, bc)
                    for k, t1 in enumerate(t1s):
                        nc.vector.scalar_tensor_tensor(
                            out=t1, in0=t1, scalar=1.0, in1=rbc,
                            op0=mybir.AluOpType.mult, op1=mybir.AluOpType.mult,
                        )
                        nc.vector.tensor_add(t1, t1, kvgb[:, k, ts(t, IT)])
                        nc.sync.dma_start(out=out_d[k, :, ts(t, IT)], in_=t1)

                packs = [(t, k) for t in range(NI) for k in range(NP4)]
                cur = {}
                pendingE = None
                pending_ep = None
                for t, k in packs:
                    if k == 0:
                        cur[t] = (
                            op.tile([P, IT], f32, tag="o0", name=f"o0_{t}"),
                            op.tile([P, IT], f32, tag="o1", name=f"o1_{t}"),
                            dp.tile([P, IT], f32, tag="dd", name=f"dd_{t}"),
                        )
                    E4 = st_exp_pack(t, k)
                    if pendingE is not None:
                        pt, pk, pE4 = pendingE
                        o0, o1, dd = cur[pt]
                        flush_pack(pt, pk, pE4, o0, o1, dd)
                        if pending_ep is not None and pk == 1:
                            epilogue(*pending_ep)
                            pending_ep = None
                        if pk == NP4 - 1:
                            pending_ep = (o0, o1, dd, pt)
                            del cur[pt]
                    pendingE = (t, k, E4)
                pt, pk, pE4 = pendingE
                o0, o1, dd = cur[pt]
                flush_pack(pt, pk, pE4, o0, o1, dd)
                if pending_ep is not None:
                    epilogue(*pending_ep)
                epilogue(o0, o1, dd, pt)

    _split_multi_waits(nc)
    return nc


def _get_nc():
    if "nc" not in _cache:
        _cache["nc"] = _build()
    return _cache["nc"]


def kernel(
    query_input, key_value_input, Wf, bf, Wg, bg, Wh, bh, gamma
):
    from concourse.bass_utils import run_bass_kernel_spmd

    B = query_input.shape[0]
    assert B == NCORES

    nc = _get_nc()

    f32 = np.float32
    par = np.zeros((P, 645), dtype=f32)
    par[:, 0:64] = Wf.T.reshape(2, P, CA).transpose(1, 0, 2).reshape(P, 64)
    par[:, 64:128] = Wg.T.reshape(2, P, CA).transpose(1, 0, 2).reshape(P, 64)
    par[:, 128:640] = Wh.T.reshape(2, P, C).transpose(1, 0, 2).reshape(P, 512)
    par[:, 640:642] = bh.reshape(2, P).T
    par[:, 642] = np.float32(gamma.reshape(-1)[0])
    par[0:CA, 643] = bf.reshape(CA)
    par[0:CA, 644] = bg.reshape(CA)
    par = np.ascontiguousarray(par)

    in_maps = []
    for b in range(B):
        in_maps.append(
            {
                "q": np.ascontiguousarray(
                    query_input[b].reshape(2, P, N), dtype=f32
                ),
                "kv": np.ascontiguousarray(
                    key_value_input[b].reshape(2, P, N), dtype=f32
                ),
                "par": par,
            }
        )

    res = run_bass_kernel_spmd(nc, in_maps, core_ids=list(range(NCORES)))
    _cache["last_result"] = res
    out = np.empty((B, C, 64, 64), dtype=f32)
    for b in range(B):
        out[b] = res.results[b]["out"].reshape(C, 64, 64)
    return out


if __name__ == "__main__":
    rng = np.random.default_rng(0)
    inputs = {
        "query_input": rng.standard_normal((8, 256, 64, 64), dtype=np.float32),
        "key_value_input": rng.standard_normal((8, 256, 64, 64), dtype=np.float32),
        "Wf": rng.standard_normal((CA, C), dtype=np.float32) * 0.06,
        "bf": rng.standard_normal((CA,), dtype=np.float32) * 0.06,
        "Wg": rng.standard_normal((CA, C), dtype=np.float32) * 0.06,
        "bg": rng.standard_normal((CA,), dtype=np.float32) * 0.06,
        "Wh": rng.standard_normal((C, C), dtype=np.float32) * 0.06,
        "bh": rng.standard_normal((C,), dtype=np.float32) * 0.06,
        "gamma": np.zeros((1,), dtype=np.float32),
    }
    out = kernel(**inputs)
    print(out.shape, out.dtype)


# revision 20
# speedup vs baseline: 1.9326x; 1.0008x over previous
"""Trainium2 Bass kernel for SAGAN-style 2D self-attention (nn_Attention2d).

Reference computation (per batch element b):
    q  = query_input[b].reshape(Cq, N)          # N = H*W = 4096, Cq = 256
    kv = key_value_input[b].reshape(C, N)       # C = 256
    fT = Wf @ q + bf        # [32, N]   (f transposed)
    g  = Wg @ kv + bg       # [32, N]
    h  = (Wh @ kv + bh).T   # [N, C]
    beta = softmax(fT.T @ g, axis=-1)           # [N, N]
    o  = beta @ h                               # [N, C]
    out[b] = gamma * o.T + kv                   # [C, N] -> [C, H, W]

Sharding: data-parallel over batch, one batch element per NeuronCore (B=8,
8 cores, no collectives).

Per-core algorithm (all layouts transposed so softmax reductions ride the
matmul path; no on-chip transposes needed):
  - projections: fT [32,N], g [32,N] (token-major), h [N,C] (token, channel)
  - loop over i-tiles (512 query tokens):
      for each j-tile (128 key tokens):
        ST  = g_j^T @ fT_i          # [128 j, 512 i] logits, PSUM
        E   = exp(ST)               # ACT, bf16 -> SBUF (no max subtraction:
                                    #  logits are bounded ~ +-13 for this op)
        o0 += h_j[:, 0:128]^T @ E   # accumulate over j in PSUM  [128 c, 512 i]
        o1 += h_j[:,128:256]^T @ E
        d  += ones^T @ E            # softmax denominator row [1, 512 i]
      s  = gamma / d                # [1, 512]
      bs = ones_col^T @ s           # PE broadcast across partitions
      out_c = o_c * bs + (kv + gamma*bh)   # DVE, then DMA out
"""

import os
import numpy as np

P = 128          # partitions
N = 4096         # tokens (H*W)
CA = 32          # attention channels
C = 256          # kv channels
IT = 512         # i-tile (query tokens per tile)
NI = N // IT     # 8
NJ = N // P      # 32
NCORES = 8

_cache = {}


def _split_multi_waits(nc, keep=1):
    """This walrus build encodes at most one sem wait per instruction
    (setupSyncWait: 'Too many sync wait commands').  Tile's sem assignment
    can attach several.  Move excess waits onto single-wait NoOps emitted
    just before the instruction on the same engine (engines execute their
    stream in order, so the waits still gate the instruction)."""
    import concourse.mybir as mybir
    import bass_rust

    for fn in nc.m.functions:
        for blk in fn.blocks:
            out = []
            for inst in blk.instructions:
                si = inst.sync_info
                if si is not None and len(si.on_wait) > keep:
                    waits = list(si.on_wait)
                    for k, w in enumerate(waits[:-keep]):
                        nop = mybir.InstNoOp(
                            name=f"{inst.name}_prewait{k}", ins=[], outs=[]
                        )
                        nop.engine = inst.engine
                        nop.sync_info = bass_rust.SyncInfo(on_wait=[w], on_update=[])
                        out.append(nop)
                    inst.sync_info = bass_rust.SyncInfo(
                        on_wait=waits[-keep:], on_update=list(si.on_update)
                    )
                out.append(inst)
            blk.instructions = out


def _build():
    import concourse.bass as bass
    import concourse.mybir as mybir
    from concourse.tile import TileContext
    from concourse.bass import ts

    f32 = mybir.dt.float32
    bf16 = mybir.dt.bfloat16
    AF = mybir.ActivationFunctionType

    nc = bass.Bass()
    q_d = nc.dram_tensor("q", [2, P, N], f32, kind="ExternalInput")
    kv_d = nc.dram_tensor("kv", [2, P, N], f32, kind="ExternalInput")
    # all small params packed into one tensor (single DMA):
    # cols [0:64]  wft (2 chunks of 32), [64:128] wgt, [128:640] wht,
    # cols [640:642] bh (2 chunks), [642] gamma (pre-broadcast by host),
    # col  [643] rows 0:32 = bf, rows 32:64 = bg
    par_d = nc.dram_tensor("par", [P, 644], f32, kind="ExternalInput")
    out_d = nc.dram_tensor("out", [2, P, N], f32, kind="ExternalOutput")

    with TileContext(nc) as tc:
        with (
            tc.tile_pool(name="const", bufs=1) as const,
            tc.tile_pool(name="big", bufs=1) as big,
        ):
            # ---- constants / small params ----
            par = const.tile([P, 644], f32)
            nc.sync.dma_start(out=par, in_=par_d[:, :])
            wft = par[:, 0:64].rearrange("p (k a) -> p k a", k=2)
            wgt = par[:, 64:128].rearrange("p (k a) -> p k a", k=2)
            wht = par[:, 128:640].rearrange("p (k a) -> p k a", k=2)
            bhs = par[:, 640:642]
            gam = par[:, 642:643]
            bfg = par[0 : 2 * CA, 643:644]  # [bf; bg] stacked
            ones_col = const.tile([P, 1], bf16)
            nc.vector.memset(ones_col, 1.0)
            # mask4: rows 0/32/64/96 are ones -> lhsT that sums the four
            # d-partial rows and broadcasts the result to all 128 partitions
            mask4 = const.tile([P, P], f32)
            nc.vector.memset(mask4, 0.0)
            for s in range(4):
                nc.vector.memset(mask4[32 * s : 32 * s + 1, :], 1.0)

            # gbh = gamma * bh  (per-partition, [128, 2])
            gbh = const.tile([P, 2], f32)
            nc.vector.tensor_scalar_mul(gbh, bhs, gam)

            # ---- big SBUF residents ----
            kv_sb = big.tile([P, 2, N], f32)
            # fT4: fT replicated into 4 row strips (partitions 32s..32s+31)
            # so 4 ST matmuls (K=32) can run concurrently via row tiling.
            # g4: strip s, block k holds g j-tile (4k+s).
            fT4 = big.tile([P, N], bf16)
            g4 = big.tile([P, (NJ // 4) * P], bf16)
            # f and g projections share one SBUF tile (rows 0:32 f, 32:64 g)
            fg_sb = big.tile([2 * CA, N], bf16)
            h = big.tile([P, NJ, C], bf16)  # [token-in-jtile, jtile, channel]

            # bf16 copies of the weights for the projection matmuls
            wgt_bf = const.tile([P, 2, CA], bf16)
            wft_bf = const.tile([P, 2, CA], bf16)
            wht_bf = const.tile([P, 2, C], bf16)
            nc.vector.tensor_copy(out=wgt_bf, in_=wgt)
            nc.vector.tensor_copy(out=wft_bf, in_=wft)
            nc.vector.tensor_copy(out=wht_bf, in_=wht)

            # ---- chunked input DMA + bf16 casts + projections ----
            # Inputs arrive in [128, 512] pieces so casts and projection
            # matmuls start as soon as the first piece lands instead of
            # waiting for the full 4 MB transfer.
            with (
                tc.tile_pool(name="qpool", bufs=1) as qpool,
                tc.tile_pool(name="projps", bufs=2, space="PSUM") as pj,
            ):
                kv_bf = qpool.tile([P, 2, N], bf16)
                q_bf = qpool.tile([P, 2, N], bf16)
                q_sb = qpool.tile([P, 2, N], f32)

                QW = 2 * IT  # 1 MB quarters
                # interleave kv and q quarters so the f projection isn't
                # gated behind the full kv transfer
                for tq in range(N // QW):
                    for k in range(2):
                        nc.sync.dma_start(
                            out=kv_sb[:, k, ts(tq, QW)], in_=kv_d[k, :, ts(tq, QW)]
                        )
                    for k in range(2):
                        nc.sync.dma_start(
                            out=q_sb[:, k, ts(tq, QW)], in_=q_d[k, :, ts(tq, QW)]
                        )

                for t in range(NI):
                    # casts: one wide instruction per input (ACT: kv, DVE: q)
                    nc.scalar.copy(
                        out=kv_bf[:, :, ts(t, IT)], in_=kv_sb[:, :, ts(t, IT)]
                    )
                    nc.vector.tensor_copy(
                        out=q_bf[:, :, ts(t, IT)], in_=q_sb[:, :, ts(t, IT)]
                    )
                    # f and g projections col-packed into one PSUM bank
                    # (f -> partitions 0:32 via col group 0, g -> 32:64 via
                    # col group 1); one ACT copy applies both biases.
                    psfg = pj.tile([2 * CA, IT], f32, tag="fg", name=f"psfg_{t}")
                    for k in range(2):
                        nc.tensor.matmul(
                            psfg[0:CA, :], lhsT=wft_bf[:, k, :],
                            rhs=q_bf[:, k, ts(t, IT)],
                            start=k == 0, stop=k == 1,
                            tile_position=(0, 0),
                        )
                        nc.tensor.matmul(
                            psfg[CA : 2 * CA, :], lhsT=wgt_bf[:, k, :],
                            rhs=kv_bf[:, k, ts(t, IT)],
                            start=k == 0, stop=k == 1,
                            tile_position=(0, CA),
                        )
                    nc.scalar.activation(
                        out=fg_sb[:, ts(t, IT)], in_=psfg, func=AF.Identity,
                        bias=bfg,
                    )
                    # h projection for the 4 j-tiles in this slice
                    for j in range(4 * t, 4 * t + 4):
                        ph = pj.tile([P, C], f32, tag="h", name=f"psh_{j}")
                        nc.tensor.matmul(
                            ph, lhsT=kv_bf[:, 0, ts(j, P)], rhs=wht_bf[:, 0, :],
                            start=True, stop=False,
                        )
                        nc.tensor.matmul(
                            ph, lhsT=kv_bf[:, 1, ts(j, P)], rhs=wht_bf[:, 1, :],
                            start=False, stop=True,
                        )
                        nc.vector.tensor_copy(out=h[:, j, :], in_=ph)

                # replicate f into the 4 fT4 row strips and scatter g
                # (rows 32:64 of fg_sb) into g4
                for s in range(4):
                    nc.gpsimd.dma_start(
                        out=fT4[CA * s : CA * (s + 1), :], in_=fg_sb[0:CA, :]
                    )
                g_blk = fg_sb[CA : 2 * CA, :].rearrange(
                    "p (k four c) -> p k four c", four=4, c=P
                )
                g4_blk = g4.rearrange("p (k c) -> p k c", c=P)
                for s in range(4):
                    nc.gpsimd.dma_start(
                        out=g4_blk[CA * s : CA * (s + 1), :, :],
                        in_=g_blk[:, :, s, :],
                    )

            # ---- main attention loop ----
            # j-tiles processed in packs of 4: the 4 ST matmuls (K=32) run
            # concurrently in the PE's four 32-row groups, writing 4 PSUM
            # banks of one [128, 4*512] tile; one ACTIVATE exps all 2048
            # columns.  PSUM banks: st4 4, o0/o1 bufs=1 = 2, d 1, bc 1 -> 8.
            with (
                tc.tile_pool(name="stps", bufs=1, space="PSUM") as stp,
                tc.tile_pool(name="ops", bufs=1, space="PSUM") as op,
                tc.tile_pool(name="dps", bufs=1, space="PSUM") as dp,
                tc.tile_pool(name="bcps", bufs=1, space="PSUM") as bcp,
                tc.tile_pool(name="epool", bufs=3) as ep,
                tc.tile_pool(name="spool", bufs=2) as sp,
                tc.tile_pool(name="outp", bufs=2) as outp,
            ):
                NP4 = NJ // 4  # packs per i-tile

                def st_exp_pack(t, k):
                    st4 = stp.tile([P, 4, IT], f32, tag="st4", name=f"st4_{t}_{k}")
                    for s in range(4):
                        nc.tensor.matmul(
                            st4[:, s, :],
                            lhsT=g4[CA * s : CA * (s + 1), ts(k, P)],
                            rhs=fT4[CA * s : CA * (s + 1), ts(t, IT)],
                            start=True, stop=True,
                            tile_position=(CA * s, 0),
                        )
                    E4 = ep.tile([P, 4, IT], bf16, tag="e4", name=f"e4_{t}_{k}")
                    nc.scalar.activation(out=E4, in_=st4, func=AF.Exp)
                    return E4

                def flush_pack(t, k, E4, o0, o1, dd):
                    first, last = k == 0, k == NP4 - 1
                    for s in range(4):
                        j = 4 * k + s
                        nc.tensor.matmul(
                            o0, lhsT=h[:, j, 0:P], rhs=E4[:, s, :],
                            start=first and s == 0, stop=last and s == 3,
                        )
                        nc.tensor.matmul(
                            o1, lhsT=h[:, j, P:C], rhs=E4[:, s, :],
                            start=first and s == 0, stop=last and s == 3,
                        )
                    # d-matmuls col-packed: 4 concurrent M=1 matmuls in the
                    # four 32-column PE groups, each writing one row (partition
                    # 32s) of the shared dd bank.
                    for s in range(4):
                        nc.tensor.matmul(
                            dd[32 * s : 32 * s + 1, :], lhsT=ones_col,
                            rhs=E4[:, s, :],
                            start=first, stop=last,
                            tile_position=(0, 32 * s),
                        )

                def epilogue(o0, o1, dd, t):
                    # copy the d bank to SBUF; the bc matmul (lhsT=mask4)
                    # sums the four partial rows (partitions 0/32/64/96) while
                    # broadcasting the result across all 128 partitions.
                    d_sb = sp.tile([P, IT], f32, tag="dsb")
                    nc.vector.tensor_copy(out=d_sb, in_=dd)
                    bc = bcp.tile([P, IT], f32, tag="bc")
                    nc.tensor.matmul(
                        bc, lhsT=mask4, rhs=d_sb, start=True, stop=True
                    )
                    t1s = []
                    for k, ok in enumerate((o0, o1)):
                        t1 = outp.tile(
                            [P, IT], f32, tag=f"out{k}", name=f"t1_{t}_{k}"
                        )
                        # release the o psum bank promptly (gamma scale only)
                        nc.vector.tensor_scalar_mul(t1, ok, gam)
                        t1s.append(t1)
                    rbc = sp.tile([P, IT], f32, tag="rbc")
                    nc.vector.reciprocal(rbc, bc)
                    for k, t1 in enumerate(t1s):
                        nc.vector.scalar_tensor_tensor(
                            out=t1, in0=t1, scalar=1.0, in1=rbc,
                            op0=mybir.AluOpType.mult, op1=mybir.AluOpType.mult,
                        )
                        # out = (t1 + gamma*bh_k) + kv
                        nc.vector.scalar_tensor_tensor(
                            out=t1, in0=t1, scalar=gbh[:, k : k + 1],
                            in1=kv_sb[:, k, ts(t, IT)],
                            op0=mybir.AluOpType.add, op1=mybir.AluOpType.add,
                        )
                        nc.sync.dma_start(out=out_d[k, :, ts(t, IT)], in_=t1)

                packs = [(t, k) for t in range(NI) for k in range(NP4)]
                cur = {}
                pendingE = None
                pending_ep = None
                for t, k in packs:
                    if k == 0:
                        cur[t] = (
                            op.tile([P, IT], f32, tag="o0", name=f"o0_{t}"),
                            op.tile([P, IT], f32, tag="o1", name=f"o1_{t}"),
                            dp.tile([P, IT], f32, tag="dd", name=f"dd_{t}"),
                        )
                    E4 = st_exp_pack(t, k)
                    if pendingE is not None:
                        pt, pk, pE4 = pendingE
                        o0, o1, dd = cur[pt]
                        flush_pack(pt, pk, pE4, o0, o1, dd)
                        if pending_ep is not None and pk == 1:
                            epilogue(*pending_ep)
                            pending_ep = None
                        if pk == NP4 - 1:
                            pending_ep = (o0, o1, dd, pt)
                            del cur[pt]
                    pendingE = (t, k, E4)
                pt, pk, pE4 = pendingE
                o0, o1, dd = cur[pt]
                flush_pack(pt, pk, pE4, o0, o1, dd)
                if pending_ep is not None:
                    epilogue(*pending_ep)
                epilogue(o0, o1, dd, pt)

    _split_multi_waits(nc)
    return nc


def _get_nc():
    if "nc" not in _cache:
        _cache["nc"] = _build()
    return _cache["nc"]


def kernel(
    query_input, key_value_input, Wf, bf, Wg, bg, Wh, bh, gamma
):
    from concourse.bass_utils import run_bass_kernel_spmd

    B = query_input.shape[0]
    assert B == NCORES

    nc = _get_nc()

    f32 = np.float32
    par = np.zeros((P, 644), dtype=f32)
    par[:, 0:64] = Wf.T.reshape(2, P, CA).transpose(1, 0, 2).reshape(P, 64)
    par[:, 64:128] = Wg.T.reshape(2, P, CA).transpose(1, 0, 2).reshape(P, 64)
    par[:, 128:640] = Wh.T.reshape(2, P, C).transpose(1, 0, 2).reshape(P, 512)
    par[:, 640:642] = bh.reshape(2, P).T
    par[:, 642] = np.float32(gamma.reshape(-1)[0])
    par[0:CA, 643] = bf.reshape(CA)
    par[CA : 2 * CA, 643] = bg.reshape(CA)
    par = np.ascontiguousarray(par)

    in_maps = []
    for b in range(B):
        in_maps.append(
            {
                "q": np.ascontiguousarray(
                    query_input[b].reshape(2, P, N), dtype=f32
                ),
                "kv": np.ascontiguousarray(
                    key_value_input[b].reshape(2, P, N), dtype=f32
                ),
                "par": par,
            }
        )

    res = run_bass_kernel_spmd(nc, in_maps, core_ids=list(range(NCORES)))
    _cache["last_result"] = res
    out = np.empty((B, C, 64, 64), dtype=f32)
    for b in range(B):
        out[b] = res.results[b]["out"].reshape(C, 64, 64)
    return out


if __name__ == "__main__":
    rng = np.random.default_rng(0)
    inputs = {
        "query_input": rng.standard_normal((8, 256, 64, 64), dtype=np.float32),
        "key_value_input": rng.standard_normal((8, 256, 64, 64), dtype=np.float32),
        "Wf": rng.standard_normal((CA, C), dtype=np.float32) * 0.06,
        "bf": rng.standard_normal((CA,), dtype=np.float32) * 0.06,
        "Wg": rng.standard_normal((CA, C), dtype=np.float32) * 0.06,
        "bg": rng.standard_normal((CA,), dtype=np.float32) * 0.06,
        "Wh": rng.standard_normal((C, C), dtype=np.float32) * 0.06,
        "bh": rng.standard_normal((C,), dtype=np.float32) * 0.06,
        "gamma": np.zeros((1,), dtype=np.float32),
    }
    out = kernel(**inputs)
    print(out.shape, out.dtype)
